# revision 18
# baseline (speedup 1.0000x reference)
"""Axial sigmoid-attention Trainium2 kernel (8 NeuronCores, SPMD).

Sharding: core = b*4 + axis*2 + half. Each core computes ONE axial-attention
axis for half the non-attended spatial extent of one batch element:
  axis 0: attend along Y within columns x in [32*half, 32*half+32)
  axis 1: attend along X within rows    y in [32*half, 32*half+32)
Positions per core: 2048 = 32 outer ("o") x 64 inner ("t", the attended dim).
The final output is the sum of the two axes' contributions (host gather).

Math (validated vs reference in numpy): sigmoid attention with RoPE.
RoPE is decomposed so no cross-partition ops are needed on device:
  logits = (c_t q)·(c_s k) + (s_t q)·(s_s k)                [A phase]
         + (c_t q)·(±s_s ksw) + (s_t q)·(∓c_s ksw)          [B phase]
with ksw = x @ Wk_swapneg (host-permuted weights), sign by rope direction d.
Each (o, head-group) logit tile is built by 4 accumulating 32-contraction
matmuls, row/col packed 8-wide on the PE array.

Mask is all-ones by construction (spec fill=ones); 1/sqrt(65) is folded into
Wv on the host. bv is zeros by spec but is still added (folded into host Wv
prep as a bias would be zero; we add it to be safe via host fold into... it
is zeros; asserted on host).
"""

import numpy as np

B, Y, X, C = 2, 64, 64, 256
M, KG = 4, 2
H, HV = 32, 32
HH = H // 2
SCALE = 1.0 / np.sqrt(H)
DEN = 1.0 / np.sqrt(65.0)
NPOS = 2048          # per-core positions
NBLK = 4             # position blocks
BLK = NPOS // NBLK   # 512 positions per block (8 outer)


# ---------------------------------------------------------------- bass program
def build_program():
    import concourse.bacc as bacc
    import concourse.mybir as mybir
    from concourse.tile import TileContext

    dt = mybir.dt
    AF = mybir.ActivationFunctionType

    nc = bacc.Bacc()

    xT = nc.declare_dram_parameter("xT", [256, NPOS], dt.float32r, isOutput=False)
    wq = nc.declare_dram_parameter("wq", [256, 1024], dt.float32r, isOutput=False)
    wk = nc.declare_dram_parameter("wk", [256, 512], dt.float32r, isOutput=False)
    wv = nc.declare_dram_parameter("wv", [256, 256], dt.float32r, isOutput=False)
    wo = nc.declare_dram_parameter("wo", [128, 8 * 256], dt.float32r, isOutput=False)
    ctab = nc.declare_dram_parameter("ctab", [128, 2048], dt.bfloat16, isOutput=False)
    stab = nc.declare_dram_parameter("stab", [128, 2048], dt.bfloat16, isOutput=False)
    outT = nc.declare_dram_parameter("outT", [256, NPOS], dt.float32, isOutput=True)

    with TileContext(nc) as tc:
        with (
            tc.tile_pool(name="fixed", bufs=1) as fixed,
            tc.tile_pool(name="qblk", bufs=2) as qblk,
            tc.tile_pool(name="kblk", bufs=2) as kblk,
            tc.tile_pool(name="wblk", bufs=3) as wpool,
            tc.tile_pool(name="vcblk", bufs=2) as vcpool,
            tc.tile_pool(name="pproj", bufs=2, space="PSUM") as pproj,
            tc.tile_pool(name="pqk", bufs=2, space="PSUM") as pqk,
            tc.tile_pool(name="pav", bufs=1, space="PSUM") as pav,
        ):
            # ---- load persistent inputs ----
            xt_sb = fixed.tile([128, 2, NPOS], dt.float32r, tag="xt")
            wq_sb = fixed.tile([128, 2, 1024], dt.float32r, tag="wq")
            wk_sb = fixed.tile([128, 2, 512], dt.float32r, tag="wk")
            wv_sb = fixed.tile([128, 2, 256], dt.float32r, tag="wv")
            wo_sb = fixed.tile([128, 8, 256], dt.float32r, tag="wo")
            ct_sb = fixed.tile([128, 2048], dt.bfloat16, tag="ct")
            st_sb = fixed.tile([128, 2048], dt.bfloat16, tag="st")
            for cc in range(2):
                nc.sync.dma_start(out=xt_sb[:, cc], in_=xT[128 * cc : 128 * cc + 128])
                nc.sync.dma_start(out=wq_sb[:, cc], in_=wq[128 * cc : 128 * cc + 128])
                nc.sync.dma_start(out=wk_sb[:, cc], in_=wk[128 * cc : 128 * cc + 128])
                nc.sync.dma_start(out=wv_sb[:, cc], in_=wv[128 * cc : 128 * cc + 128])
            nc.sync.dma_start(out=wo_sb[:], in_=wo[:].rearrange("p (c n) -> p c n", c=8))
            nc.sync.dma_start(out=ct_sb[:], in_=ctab[:])
            nc.sync.dma_start(out=st_sb[:], in_=stab[:])

            # PE vector-clock warmup: one tiny matmul per input DMA so no real
            # matmul ever needs more than one fresh DMA-semaphore wait
            # (walrus allows a single sync-wait per Matmult).
            warm = pqk.tile([128, 1024], dt.float32, tag="qk", name="warm")
            for i, sl in enumerate(
                [xt_sb[:, 0, 0:8], xt_sb[:, 1, 0:8], wq_sb[:, 0, 0:8], wq_sb[:, 1, 0:8],
                 wk_sb[:, 0, 0:8], wk_sb[:, 1, 0:8], wv_sb[:, 0, 0:8], wv_sb[:, 1, 0:8],
                 wo_sb[:, 0, 0:8], ct_sb[:, 0:8], st_sb[:, 0:8]]
            ):
                nc.tensor.matmul(warm[0:8, 8 * i : 8 * i + 8], lhsT=sl, rhs=sl,
                                 start=True, stop=True)
            # same for DVE / GPSIMD consumers of DMA'd tables
            wscr = fixed.tile([128, 32], dt.bfloat16, tag="wscr")
            nc.vector.tensor_copy(wscr[:, 0:8], ct_sb[:, 0:8])
            nc.vector.tensor_copy(wscr[:, 8:16], st_sb[:, 0:8])
            nc.gpsimd.tensor_copy(wscr[:, 16:24], ct_sb[:, 0:8])
            nc.gpsimd.tensor_copy(wscr[:, 24:32], st_sb[:, 0:8])

            for blk in range(NBLK):
                p0 = blk * BLK
                # ================= projections =================
                # q: per (m, cht): psum [128, 512] -> bf16 q_sb[cht][:, m*512:...]
                q_sb = [qblk.tile([128, M * BLK], dt.bfloat16, tag=f"q{c}", name=f"q{c}_{blk}") for c in range(2)]
                for m in range(M):
                    for cht in range(2):
                        ps = pproj.tile([128, BLK], dt.float32, tag="proj")
                        for cc in range(2):
                            nc.tensor.matmul(
                                ps[:],
                                lhsT=wq_sb[:, cc, m * 256 + cht * 128 : m * 256 + (cht + 1) * 128],
                                rhs=xt_sb[:, cc, p0 : p0 + BLK],
                                start=(cc == 0),
                                stop=(cc == 1),
                            )
                        # drain PSUM -> SBUF bf16 (DVE)
                        nc.vector.tensor_copy(q_sb[cht][:, m * BLK : (m + 1) * BLK], ps[:])

                # rope tables broadcast over m: [128, m, 512]
                cq_sb = [qblk.tile([128, M * BLK], dt.bfloat16, tag=f"cq{c}", name=f"cq{c}_{blk}") for c in range(2)]
                sq_sb = [qblk.tile([128, M * BLK], dt.bfloat16, tag=f"sq{c}", name=f"sq{c}_{blk}") for c in range(2)]
                for cht in range(2):
                    nc.vector.tensor_mul(cq_sb[cht][:], q_sb[cht][:], ct_sb[:])
                    nc.vector.tensor_mul(sq_sb[cht][:], q_sb[cht][:], st_sb[:])

                # k_ext: 4 ch-tiles (e, d-half): psum -> bf16
                ke_sb = [kblk.tile([128, BLK], dt.bfloat16, tag=f"ke{c}", name=f"ke{c}_{blk}") for c in range(4)]
                for cht in range(4):
                    ps = pproj.tile([128, BLK], dt.float32, tag="proj")
                    for cc in range(2):
                        nc.tensor.matmul(
                            ps[:],
                            lhsT=wk_sb[:, cc, cht * 128 : (cht + 1) * 128],
                            rhs=xt_sb[:, cc, p0 : p0 + BLK],
                            start=(cc == 0),
                            stop=(cc == 1),
                        )
                    nc.scalar.copy(ke_sb[cht][:], ps[:])

                # products + combines -> Rk (pairs with cq), Rpk (pairs with sq)
                # d=0 tiles (cht even): Rk = C*k + S*ksw ; Rpk = S*k - C*ksw
                # d=1 tiles (cht odd):  Rk = C*k - S*ksw ; Rpk = S*k + C*ksw
                rk_sb = [kblk.tile([128, BLK], dt.bfloat16, tag=f"rk{c}", name=f"rk{c}_{blk}") for c in range(2)]
                rpk_sb = [kblk.tile([128, BLK], dt.bfloat16, tag=f"rpk{c}", name=f"rpk{c}_{blk}") for c in range(2)]
                for dh in range(2):  # d half = cht parity
                    pk = kblk.tile([128, BLK], dt.bfloat16, tag="pk")
                    psw = kblk.tile([128, BLK], dt.bfloat16, tag="psw")
                    nc.vector.tensor_mul(pk[:], ke_sb[dh][:], ct_sb[:, :BLK])
                    nc.vector.tensor_mul(psw[:], ke_sb[2 + dh][:], st_sb[:, :BLK])
                    if dh == 0:
                        nc.gpsimd.tensor_add(rk_sb[dh][:], pk[:], psw[:])
                    else:
                        nc.gpsimd.tensor_sub(rk_sb[dh][:], pk[:], psw[:])
                    pk2 = kblk.tile([128, BLK], dt.bfloat16, tag="pk2")
                    psw2 = kblk.tile([128, BLK], dt.bfloat16, tag="psw2")
                    nc.vector.tensor_mul(pk2[:], ke_sb[dh][:], st_sb[:, :BLK])
                    nc.vector.tensor_mul(psw2[:], ke_sb[2 + dh][:], ct_sb[:, :BLK])
                    if dh == 0:
                        nc.gpsimd.tensor_sub(rpk_sb[dh][:], pk2[:], psw2[:])
                    else:
                        nc.gpsimd.tensor_add(rpk_sb[dh][:], pk2[:], psw2[:])

                # vT: per o-pair: psum [128, 256] = [(o2, t) pos, 256 ch] -> bf16
                vt_sb = [kblk.tile([128, 256], dt.bfloat16, tag=f"vt{i}", name=f"vt{i}_{blk}") for i in range(4)]
                for op2 in range(4):
                    pp = p0 + op2 * 128
                    ps = pproj.tile([128, BLK], dt.float32, tag="proj", name=f"vtps{op2}_{blk}")
                    for cc in range(2):
                        nc.tensor.matmul(
                            ps[:, :256],
                            lhsT=xt_sb[:, cc, pp : pp + 128],
                            rhs=wv_sb[:, cc],
                            start=(cc == 0),
                            stop=(cc == 1),
                        )
                    nc.scalar.copy(vt_sb[op2][:], ps[:, :256])

                # ================= attention =================
                # HW rule: matmuls from different row-groups must not write the
                # same (psum bank, col-group). So each g4 row-group gets its own
                # psum bank: qkA holds g4 0-1 (one bank each), qkB g4 2-3.
                # Two op2's share a bank (same row-group) to keep sigmoid wide.
                vc_sb = [vcpool.tile([128, 8 * 256], dt.float32r, tag=f"vc{q}", name=f"vc{q}_{blk}") for q in range(2)]
                for op2p in range(2):
                    w_q = []
                    for quad in range(2):
                        qkA = pqk.tile([128, 1024], dt.float32, tag="qk", name=f"qkA{op2p}_{quad}_{blk}")
                        qkB = pqk.tile([128, 1024], dt.float32, tag="qk", name=f"qkB{op2p}_{quad}_{blk}")
                        for op2l in range(2):
                            for oo in range(2):
                                o = (op2p * 2 + op2l) * 2 + oo
                                for g4 in range(4):
                                    qk = qkA if g4 < 2 else qkB
                                    foff = 512 * (g4 % 2) + 256 * op2l
                                    for ph, (kt, ut) in enumerate(((rk_sb, cq_sb), (rpk_sb, sq_sb))):
                                        nc.tensor.matmul(
                                            qk[64 * oo : 64 * oo + 64, foff : foff + 256],
                                            lhsT=kt[quad][32 * g4 : 32 * g4 + 32, o * 64 : (o + 1) * 64],
                                            rhs=ut[quad][:]
                                            .rearrange("p (m t) -> p m t", m=M)[
                                                32 * g4 : 32 * g4 + 32, :, o * 64 : (o + 1) * 64
                                            ],
                                            start=(ph == 0),
                                            stop=(ph == 1),
                                            tile_position=(32 * g4, 64 * oo),
                                        )
                        w_sb = wpool.tile([128, 2048], dt.bfloat16, tag="w", name=f"w{op2p}_{quad}_{blk}")
                        nc.scalar.activation(w_sb[:, 0:1024], qkA[:], AF.Sigmoid, scale=SCALE)
                        nc.scalar.activation(w_sb[:, 1024:2048], qkB[:], AF.Sigmoid, scale=SCALE)
                        w_q.append(w_sb)
                    for op2l in range(2):
                        op2 = op2p * 2 + op2l
                        # av banks: bank0 = oo0, bank1 = oo1 (one row-group per bank)
                        av = pav.tile([128, 1024], dt.float32, tag="av", name=f"av{op2}_{blk}")
                        for oo in range(2):
                            for quad in range(2):
                                for g4 in range(4):
                                    woff = 1024 * (g4 // 2) + 512 * (g4 % 2) + 256 * op2l
                                    nc.tensor.matmul(
                                        av[32 * g4 : 32 * g4 + 32,
                                           512 * oo + 256 * quad : 512 * oo + 256 * quad + 256],
                                        lhsT=vt_sb[op2][
                                            64 * oo : 64 * oo + 64,
                                            32 * (quad * 4 + g4) : 32 * (quad * 4 + g4) + 32,
                                        ],
                                        rhs=w_q[quad][64 * oo : 64 * oo + 64, woff : woff + 256],
                                        start=True,
                                        stop=True,
                                        tile_position=(64 * oo, 32 * g4),
                                    )
                        av4 = av[:].rearrange("p (oo q mt) -> p oo q mt", oo=2, q=2)
                        for quad in range(2):
                            nc.vector.tensor_copy(
                                vc_sb[quad][:, op2 * 512 : (op2 + 1) * 512]
                                .rearrange("p (oo mt) -> p oo mt", oo=2),
                                av4[:, :, quad, :],
                            )

                # ================= output projection =================
                for och in range(2):
                    ps = pproj.tile([128, BLK], dt.float32, tag="proj", name=f"ops{och}_{blk}")
                    for ch in range(8):
                        m, quad = ch // 2, ch % 2
                        nc.tensor.matmul(
                            ps[:],
                            lhsT=wo_sb[:, ch, och * 128 : (och + 1) * 128],
                            rhs=vc_sb[quad][:]
                            .rearrange("p (o m t) -> p o m t", o=8, m=M)[:, :, m, :],
                            start=(ch == 0),
                            stop=(ch == 7),
                        )
                    o_sb = wpool.tile([128, BLK], dt.float32, tag="osb", name=f"osb{och}_{blk}")
                    nc.vector.tensor_copy(o_sb[:], ps[:])
                    nc.sync.dma_start(
                        out=outT[och * 128 : (och + 1) * 128, p0 : p0 + BLK], in_=o_sb[:]
                    )
    nc.compile()
    return nc


# ---------------------------------------------------------------- host side
def _rope_tables(pos_arr, rope_freq):
    scaling = np.pi / np.stack([np.linspace(1, 30, HH), np.linspace(0.1, 1, HH)], -1)
    freq = rope_freq * scaling.astype(np.float32)
    phi = (pos_arr[:, None, :] * freq[None, :, :]).sum(-1)   # [64, HH]
    cs, sn = np.cos(phi), np.sin(phi)
    Ct = np.repeat(cs.T, 2, axis=0).astype(np.float32)        # [32, 64] rows h
    St = np.repeat(sn.T, 2, axis=0).astype(np.float32)
    return np.tile(Ct, (4, 32)), np.tile(St, (4, 32))         # [128, 2048]


def _build_weights(Wq, Wk, Wv, Wo, axis):
    Wq_a = Wq[:, 2 * axis : 2 * axis + 2]                     # [C, d, v, M, KG, H]
    wq = np.transpose(Wq_a, (0, 3, 1, 2, 4, 5)).reshape(C, 1024)  # (m, d, v, kg, h)
    Wk_a = Wk[:, 2 * axis : 2 * axis + 2].reshape(C, 2, 2, KG, HH, 2)
    Wk_swap = np.stack([Wk_a[..., 1], -Wk_a[..., 0]], -1)
    wk = np.concatenate([Wk_a.reshape(C, 256), Wk_swap.reshape(C, 256)], 1)  # (e,d,v,kg,h)
    wv = (Wv[:, 2 * axis : 2 * axis + 2].reshape(C, 256) * DEN).astype(np.float32)
    Wo_a = Wo[2 * axis : 2 * axis + 2]                        # [d, v, M, KG, HV, C]
    Wo_perm = np.transpose(Wo_a, (2, 0, 1, 3, 4, 5)).reshape(M * 256, C)  # (m, c, f)
    wo = Wo_perm.reshape(8, 128, 256).transpose(1, 0, 2).reshape(128, 8 * 256)
    return (np.ascontiguousarray(wq, np.float32), np.ascontiguousarray(wk, np.float32),
            np.ascontiguousarray(wv), np.ascontiguousarray(wo, np.float32))


def prepare_in_maps(x, Wq, Wk, Wv, bv, Wo, rope_freq, ypos, xpos, mask):
    import ml_dtypes

    assert np.abs(bv).max() == 0.0, "kernel assumes bv == 0 (spec fill=zeros)"
    Ct0, St0 = _rope_tables(ypos, rope_freq)
    Ct1, St1 = _rope_tables(xpos, rope_freq)
    waxis = [_build_weights(Wq, Wk, Wv, Wo, a) for a in range(2)]
    in_maps = []
    for core in range(8):
        b, axis, half = core // 4, (core // 2) % 2, core % 2
        wq, wk, wv, wo = waxis[axis]
        if axis == 0:
            blkx = x[b, :, 32 * half : 32 * half + 32, :]     # [Y, 32, C]
            xT = np.transpose(blkx, (2, 1, 0)).reshape(C, NPOS)  # (c, o=x, t=y)
            Ct, St = Ct0, St0
        else:
            blkx = x[b, 32 * half : 32 * half + 32, :, :]     # [32, X, C]
            xT = np.transpose(blkx, (2, 0, 1)).reshape(C, NPOS)  # (c, o=y, t=x)
            Ct, St = Ct1, St1
        in_maps.append(
            dict(
                xT=np.ascontiguousarray(xT, np.float32),
                wq=wq, wk=wk, wv=wv, wo=wo,
                ctab=Ct.astype(ml_dtypes.bfloat16),
                stab=St.astype(ml_dtypes.bfloat16),
            )
        )
    return in_maps


def gather_output(results):
    out = np.zeros((B, Y, X, C), np.float32)
    for core in range(8):
        b, axis, half = core // 4, (core // 2) % 2, core % 2
        outT = results[core]["outT"].reshape(C, 32, 64)
        if axis == 0:
            out[b, :, 32 * half : 32 * half + 32, :] += np.transpose(outT, (2, 1, 0))
        else:
            out[b, 32 * half : 32 * half + 32, :, :] += np.transpose(outT, (1, 2, 0))
    return out


_CACHED = {}


def kernel(x, Wq, Wk, Wv, bv, Wo, rope_freq, ypos, xpos, mask):
    from concourse.bass_utils import run_bass_kernel_spmd

    x, Wq, Wk, Wv, bv, Wo, rope_freq, ypos, xpos = (
        np.asarray(a, np.float32) for a in (x, Wq, Wk, Wv, bv, Wo, rope_freq, ypos, xpos)
    )
    in_maps = prepare_in_maps(x, Wq, Wk, Wv, bv, Wo, rope_freq, ypos, xpos, mask)
    if "nc" not in _CACHED:
        _CACHED["nc"] = build_program()
    res = run_bass_kernel_spmd(_CACHED["nc"], in_maps, core_ids=list(range(8)))
    return gather_output(res.results)


# revision 33
# speedup vs baseline: 1.3524x; 1.3524x over previous
"""Axial sigmoid-attention Trainium2 kernel (8 NeuronCores, SPMD) — v5.

Sharding: core = b*4 + axis*2 + half; each core runs ONE axis over half the
non-attended spatial extent of one batch element (2048 positions = 32 outer
x 64 attended). Host gathers and sums the two axes' contributions.

RoPE decomposition (no cross-partition ops): logits accumulate in PSUM as
  A: (c_t q)·(c_s k) + (s_t q)·(s_s k)
  B: (c_t q)·(±s_s ksw) + (s_t q)·(∓c_s ksw),  ksw = x @ Wk_swapneg (host).

Schedule: per-block units are software-pipelined — attention(b) is emitted
interleaved with projections(b+1) and out-projection(b-1) so the PE always
has independent matmul work during sigmoid/drain latencies (keeps HAM warm).
PSUM rule respected throughout: matmuls from different row-groups never
write the same (bank, col-group).
"""

import numpy as np

B, Y, X, C = 2, 64, 64, 256
M, KG = 4, 2
H, HV = 32, 32
HH = H // 2
SCALE = 1.0 / np.sqrt(H)
DEN = 1.0 / np.sqrt(65.0)
NPOS = 2048
NBLK = 4
BLK = NPOS // NBLK



# ---------------------------------------------------------------- bass program
def build_program():
    import concourse.bacc as bacc
    import concourse.mybir as mybir
    from concourse.tile import TileContext

    dt = mybir.dt
    AF = mybir.ActivationFunctionType

    nc = bacc.Bacc()

    xT = nc.declare_dram_parameter("xT", [256, NPOS], dt.float32r, isOutput=False)
    wq = nc.declare_dram_parameter("wq", [256, 1024], dt.float32r, isOutput=False)
    wk = nc.declare_dram_parameter("wk", [256, 512], dt.float32r, isOutput=False)
    wv = nc.declare_dram_parameter("wv", [256, 256], dt.float32r, isOutput=False)
    wo = nc.declare_dram_parameter("wo", [128, 8 * 256], dt.float32r, isOutput=False)
    ctab = nc.declare_dram_parameter("ctab", [128, 2048], dt.bfloat16, isOutput=False)
    stab = nc.declare_dram_parameter("stab", [128, 2048], dt.bfloat16, isOutput=False)
    outT = nc.declare_dram_parameter("outT", [256, NPOS], dt.float32, isOutput=True)

    with TileContext(nc) as tc:
        with (
            tc.tile_pool(name="fixed", bufs=1) as fixed,
            tc.tile_pool(name="qblk", bufs=2) as qblk,
            tc.tile_pool(name="kblk", bufs=2) as kblk,
            tc.tile_pool(name="wblk", bufs=3) as wpool,
            tc.tile_pool(name="vcblk", bufs=2) as vcpool,
            tc.tile_pool(name="pproj", bufs=2, space="PSUM") as pproj,
            tc.tile_pool(name="pqk", bufs=3, space="PSUM") as pqk,
        ):
            xt_sb = fixed.tile([128, 2, NPOS], dt.float32r, tag="xt")
            wq_sb = fixed.tile([128, 2, 1024], dt.float32r, tag="wq")
            wk_sb = fixed.tile([128, 2, 512], dt.float32r, tag="wk")
            wv_sb = fixed.tile([128, 2, 256], dt.float32r, tag="wv")
            wo_sb = fixed.tile([128, 8, 256], dt.float32r, tag="wo")
            ct_sb = fixed.tile([128, 2048], dt.bfloat16, tag="ct")
            st_sb = fixed.tile([128, 2048], dt.bfloat16, tag="st")
            # weights first, then x chunked per block so block-0 proj starts early
            for cc in range(2):
                nc.sync.dma_start(out=wq_sb[:, cc, 0:512], in_=wq[128 * cc : 128 * cc + 128, 0:512])
                nc.sync.dma_start(
                    out=xt_sb[:, cc, 0:BLK],
                    in_=xT[128 * cc : 128 * cc + 128, 0:BLK],
                )
            for cc in range(2):
                nc.sync.dma_start(out=wq_sb[:, cc, 512:1024], in_=wq[128 * cc : 128 * cc + 128, 512:1024])
                nc.gpsimd.dma_start(out=wk_sb[:, cc], in_=wk[128 * cc : 128 * cc + 128])
                nc.gpsimd.dma_start(out=wv_sb[:, cc], in_=wv[128 * cc : 128 * cc + 128])
            nc.gpsimd.dma_start(out=ct_sb[:], in_=ctab[:])
            nc.gpsimd.dma_start(out=st_sb[:], in_=stab[:])
            nc.gpsimd.dma_start(out=wo_sb[:], in_=wo[:].rearrange("p (c n) -> p c n", c=8))
            for b in range(1, NBLK):
                for cc in range(2):
                    nc.scalar.dma_start(
                        out=xt_sb[:, cc, b * BLK : (b + 1) * BLK],
                        in_=xT[128 * cc : 128 * cc + 128, b * BLK : (b + 1) * BLK],
                    )

            # vector-clock warmups: each engine observes every input-DMA lane
            # via ops that depend on exactly one DMA (HW allows one sync wait
            # per instruction).
            warm = pqk.tile([128, 512], dt.float32, tag="qk2", name="warm")
            _wi = [0]

            def warm_touch(sl):
                i = _wi[0]; _wi[0] += 1
                nc.tensor.matmul(warm[0:8, 8 * i : 8 * i + 8], lhsT=sl, rhs=sl,
                                 start=True, stop=True)

            # stage 1: only what the first q-units need
            for sl in [wq_sb[:, 0, 0:8], wq_sb[:, 1, 0:8],
                       xt_sb[:, 0, 0:8], xt_sb[:, 1, 0:8]]:
                warm_touch(sl)
            wscr = fixed.tile([128, 32], dt.bfloat16, tag="wscr")
            nc.vector.tensor_copy(wscr[:, 0:8], ct_sb[:, 0:8])
            nc.vector.tensor_copy(wscr[:, 8:16], st_sb[:, 0:8])
            nc.gpsimd.tensor_copy(wscr[:, 16:24], ct_sb[:, 0:8])
            nc.gpsimd.tensor_copy(wscr[:, 24:32], st_sb[:, 0:8])

            S = {}  # per-block live tiles

            def proj_units(blk):
                """Generator of emit-callbacks for block `blk` projections."""
                p0 = blk * BLK
                st = S[blk] = {}
                st["q"] = [qblk.tile([128, M * BLK], dt.bfloat16, tag=f"q{c}", name=f"q{c}_{blk}") for c in range(2)]
                st["cq"] = [qblk.tile([128, M * BLK], dt.bfloat16, tag=f"cq{c}", name=f"cq{c}_{blk}") for c in range(2)]
                st["sq"] = [qblk.tile([128, M * BLK], dt.bfloat16, tag=f"sq{c}", name=f"sq{c}_{blk}") for c in range(2)]
                st["ke"] = [kblk.tile([128, BLK], dt.bfloat16, tag=f"ke{c}", name=f"ke{c}_{blk}") for c in range(4)]
                st["rk"] = [kblk.tile([128, BLK], dt.bfloat16, tag=f"rk{c}", name=f"rk{c}_{blk}") for c in range(2)]
                st["rpk"] = [kblk.tile([128, BLK], dt.bfloat16, tag=f"rpk{c}", name=f"rpk{c}_{blk}") for c in range(2)]
                st["vt"] = [kblk.tile([128, 256], dt.bfloat16, tag=f"vt{i}", name=f"vt{i}_{blk}") for i in range(4)]
                st["vc"] = [vcpool.tile([128, 8 * 256], dt.float32r, tag=f"vc{q}", name=f"vc{q}_{blk}") for q in range(2)]

                def q_unit(m, cht):
                    def emit():
                        ps = pproj.tile([128, BLK], dt.float32, tag="proj", name=f"qp{m}{cht}_{blk}")
                        for cc in range(2):
                            nc.tensor.matmul(
                                ps[:],
                                lhsT=wq_sb[:, cc, m * 256 + cht * 128 : m * 256 + (cht + 1) * 128],
                                rhs=xt_sb[:, cc, p0 : p0 + BLK],
                                start=(cc == 0), stop=(cc == 1),
                            )
                        if cht == 0:
                            nc.vector.tensor_copy(st["q"][cht][:, m * BLK : (m + 1) * BLK], ps[:])
                        else:
                            nc.scalar.copy(st["q"][cht][:, m * BLK : (m + 1) * BLK], ps[:])
                    return emit

                def k_unit(cht):
                    def emit():
                        ps = pproj.tile([128, BLK], dt.float32, tag="proj", name=f"kp{cht}_{blk}")
                        for cc in range(2):
                            nc.tensor.matmul(
                                ps[:],
                                lhsT=wk_sb[:, cc, cht * 128 : (cht + 1) * 128],
                                rhs=xt_sb[:, cc, p0 : p0 + BLK],
                                start=(cc == 0), stop=(cc == 1),
                            )
                        nc.scalar.copy(st["ke"][cht][:], ps[:])
                    return emit

                def vt_unit(op2):
                    def emit():
                        pp = p0 + op2 * 128
                        ps = pproj.tile([128, BLK], dt.float32, tag="proj", name=f"vtp{op2}_{blk}")
                        for cc in range(2):
                            nc.tensor.matmul(
                                ps[:, :256],
                                lhsT=xt_sb[:, cc, pp : pp + 128],
                                rhs=wv_sb[:, cc],
                                start=(cc == 0), stop=(cc == 1),
                            )
                        nc.scalar.copy(st["vt"][op2][:], ps[:, :256])
                    return emit

                def rope_unit(cht):
                    def emit():
                        nc.vector.tensor_mul(st["cq"][cht][:], st["q"][cht][:], ct_sb[:])
                        nc.vector.tensor_mul(st["sq"][cht][:], st["q"][cht][:], st_sb[:])
                    return emit

                def kprod_unit(dh):
                    def emit():
                        pk = kblk.tile([128, BLK], dt.bfloat16, tag="pk", name=f"pk{dh}_{blk}")
                        psw = kblk.tile([128, BLK], dt.bfloat16, tag="psw", name=f"psw{dh}_{blk}")
                        nc.vector.tensor_mul(pk[:], st["ke"][dh][:], ct_sb[:, :BLK])
                        nc.vector.tensor_mul(psw[:], st["ke"][2 + dh][:], st_sb[:, :BLK])
                        if dh == 0:
                            nc.vector.tensor_add(st["rk"][dh][:], pk[:], psw[:])
                        else:
                            nc.vector.tensor_sub(st["rk"][dh][:], pk[:], psw[:])
                        pk2 = kblk.tile([128, BLK], dt.bfloat16, tag="pk2", name=f"pk2{dh}_{blk}")
                        psw2 = kblk.tile([128, BLK], dt.bfloat16, tag="psw2", name=f"psw2{dh}_{blk}")
                        nc.vector.tensor_mul(pk2[:], st["ke"][dh][:], st_sb[:, :BLK])
                        nc.vector.tensor_mul(psw2[:], st["ke"][2 + dh][:], ct_sb[:, :BLK])
                        if dh == 0:
                            nc.vector.tensor_sub(st["rpk"][dh][:], pk2[:], psw2[:])
                        else:
                            nc.vector.tensor_add(st["rpk"][dh][:], pk2[:], psw2[:])
                    return emit

                units = [q_unit(m, cht) for m in range(M) for cht in range(2)]
                units.append(rope_unit(0))
                units.append(rope_unit(1))
                units += [k_unit(c) for c in range(4)]
                units.append(kprod_unit(0))
                units.append(kprod_unit(1))
                units += [vt_unit(i) for i in range(4)]
                return units

            def attn_units(blk):
                st = S[blk]
                units = []

                def qk_unit(op2p, quad):
                    def emit():
                        qkA = pqk.tile([128, 1024], dt.float32, tag="qk2", name=f"qkA{op2p}_{quad}_{blk}")
                        qkB = pqk.tile([128, 1024], dt.float32, tag="qk2", name=f"qkB{op2p}_{quad}_{blk}")
                        qkg = [qkA, qkA, qkB, qkB]
                        for g4 in range(4):
                            for op2l in range(2):
                                for oo in range(2):
                                    o = (op2p * 2 + op2l) * 2 + oo
                                    for ph, (kt, ut) in enumerate(((st["rk"], st["cq"]), (st["rpk"], st["sq"]))):
                                        foff = 512 * (g4 % 2) + 256 * op2l
                                        nc.tensor.matmul(
                                            qkg[g4][64 * oo : 64 * oo + 64, foff : foff + 256],
                                            lhsT=kt[quad][32 * g4 : 32 * g4 + 32, o * 64 : (o + 1) * 64],
                                            rhs=ut[quad][:]
                                            .rearrange("p (m t) -> p m t", m=M)[
                                                32 * g4 : 32 * g4 + 32, :, o * 64 : (o + 1) * 64
                                            ],
                                            start=(ph == 0), stop=(ph == 1),
                                            tile_position=(32 * g4, 64 * oo),
                                        )
                        w_sb = wpool.tile([128, 2048], dt.bfloat16, tag="w", name=f"w{op2p}_{quad}_{blk}")
                        for bk in range(2):
                            nc.scalar.activation(w_sb[:, 512 * bk : 512 * bk + 512],
                                                 qkA[:, 512 * bk : 512 * bk + 512], AF.Sigmoid, scale=SCALE)
                            nc.scalar.activation(w_sb[:, 1024 + 512 * bk : 1536 + 512 * bk],
                                                 qkB[:, 512 * bk : 512 * bk + 512], AF.Sigmoid, scale=SCALE)
                        st[f"w{op2p}_{quad}"] = w_sb
                    return emit

                def av_unit(op2p, op2l):
                    def emit():
                        op2 = op2p * 2 + op2l
                        av = pqk.tile([128, 1024], dt.float32, tag="qk2", name=f"av{op2}_{blk}")
                        for quad in range(2):
                            for oo in range(2):
                                for g4 in range(4):
                                    woff = 1024 * (g4 // 2) + 512 * (g4 % 2) + 256 * op2l
                                    nc.tensor.matmul(
                                        av[32 * g4 : 32 * g4 + 32,
                                           512 * oo + 256 * quad : 512 * oo + 256 * quad + 256],
                                        lhsT=st["vt"][op2][
                                            64 * oo : 64 * oo + 64,
                                            32 * (quad * 4 + g4) : 32 * (quad * 4 + g4) + 32,
                                        ],
                                        rhs=st[f"w{op2p}_{quad}"][64 * oo : 64 * oo + 64, woff : woff + 256],
                                        start=True, stop=True,
                                        tile_position=(64 * oo, 32 * g4),
                                    )
                        av4 = av[:].rearrange("p (oo q mt) -> p oo q mt", oo=2, q=2)
                        for quad in range(2):
                            nc.vector.tensor_copy(
                                st["vc"][quad][:, op2 * 512 : (op2 + 1) * 512]
                                .rearrange("p (oo mt) -> p oo mt", oo=2),
                                av4[:, :, quad, :],
                            )
                    return emit

                for op2p in range(2):
                    units.append(qk_unit(op2p, 0))
                    units.append(qk_unit(op2p, 1))
                    units.append(av_unit(op2p, 0))
                    units.append(av_unit(op2p, 1))
                return units

            def outproj_units(blk):
                st = S[blk]

                def unit(och):
                    def emit():
                        ps = pproj.tile([128, BLK], dt.float32, tag="proj", name=f"ops{och}_{blk}")
                        for ch in range(8):
                            m, quad = ch // 2, ch % 2
                            nc.tensor.matmul(
                                ps[:],
                                lhsT=wo_sb[:, ch, och * 128 : (och + 1) * 128],
                                rhs=st["vc"][quad][:]
                                .rearrange("p (o m t) -> p o m t", o=8, m=M)[:, :, m, :],
                                start=(ch == 0), stop=(ch == 7),
                            )
                        o_sb = wpool.tile([128, BLK], dt.float32, tag="osb", name=f"osb{och}_{blk}")
                        nc.vector.tensor_copy(o_sb[:], ps[:])
                        nc.sync.dma_start(
                            out=outT[och * 128 : (och + 1) * 128, blk * BLK : (blk + 1) * BLK],
                            in_=o_sb[:],
                        )
                    return emit

                return [unit(0), unit(1)]

            def interleave(primary, fillers):
                """Emit primary units with filler units spread between them."""
                if not primary:
                    for f in fillers:
                        f()
                    return
                k = len(fillers)
                n = len(primary)
                fi = 0
                for i, p in enumerate(primary):
                    p()
                    take = (k * (i + 1)) // n - fi
                    for _ in range(take):
                        fillers[fi]()
                        fi += 1

            # prologue: block-0 projections, warming late tensors just in time
            p0units = proj_units(0)
            # unit order: 8 q-units, 2 rope, 4 k-units, 2 kprod, 4 vt-units
            for i, u in enumerate(p0units):
                if i == 4:
                    warm_touch(wq_sb[:, 0, 512:520]); warm_touch(wq_sb[:, 1, 512:520])
                if i == 10:
                    warm_touch(wk_sb[:, 0, 0:8]); warm_touch(wk_sb[:, 1, 0:8])
                if i == 16:
                    warm_touch(wv_sb[:, 0, 0:8]); warm_touch(wv_sb[:, 1, 0:8])
                u()
            warm_touch(wo_sb[:, 0, 0:8])
            warm_touch(ct_sb[:, 0:8]); warm_touch(st_sb[:, 0:8])
            for blk in range(NBLK):
                fillers = []
                if blk + 1 < NBLK:
                    for cc in range(2):
                        warm_touch(xt_sb[:, cc, (blk + 1) * BLK : (blk + 1) * BLK + 8])
                    fillers += proj_units(blk + 1)
                if blk - 1 >= 0:
                    fillers += outproj_units(blk - 1)
                interleave(attn_units(blk), fillers)
            for u in outproj_units(NBLK - 1):
                u()

    nc.compile()
    return nc


# ---------------------------------------------------------------- host side
def _rope_tables(pos_arr, rope_freq):
    scaling = np.pi / np.stack([np.linspace(1, 30, HH), np.linspace(0.1, 1, HH)], -1)
    freq = rope_freq * scaling.astype(np.float32)
    phi = (pos_arr[:, None, :] * freq[None, :, :]).sum(-1)   # [64, HH]
    cs, sn = np.cos(phi), np.sin(phi)
    Ct = np.repeat(cs.T, 2, axis=0).astype(np.float32)        # [32, 64] rows h
    St = np.repeat(sn.T, 2, axis=0).astype(np.float32)
    return np.tile(Ct, (4, 32)), np.tile(St, (4, 32))         # [128, 2048]


def _build_weights(Wq, Wk, Wv, Wo, axis):
    Wq_a = Wq[:, 2 * axis : 2 * axis + 2]                     # [C, d, v, M, KG, H]
    wq = np.transpose(Wq_a, (0, 3, 1, 2, 4, 5)).reshape(C, 1024)  # (m, d, v, kg, h)
    Wk_a = Wk[:, 2 * axis : 2 * axis + 2].reshape(C, 2, 2, KG, HH, 2)
    Wk_swap = np.stack([Wk_a[..., 1], -Wk_a[..., 0]], -1)
    wk = np.concatenate([Wk_a.reshape(C, 256), Wk_swap.reshape(C, 256)], 1)  # (e,d,v,kg,h)
    wv = (Wv[:, 2 * axis : 2 * axis + 2].reshape(C, 256) * DEN).astype(np.float32)
    Wo_a = Wo[2 * axis : 2 * axis + 2]                        # [d, v, M, KG, HV, C]
    Wo_perm = np.transpose(Wo_a, (2, 0, 1, 3, 4, 5)).reshape(M * 256, C)  # (m, c, f)
    wo = Wo_perm.reshape(8, 128, 256).transpose(1, 0, 2).reshape(128, 8 * 256)
    return (np.ascontiguousarray(wq, np.float32), np.ascontiguousarray(wk, np.float32),
            np.ascontiguousarray(wv), np.ascontiguousarray(wo, np.float32))


def prepare_in_maps(x, Wq, Wk, Wv, bv, Wo, rope_freq, ypos, xpos, mask):
    import ml_dtypes

    assert np.abs(bv).max() == 0.0, "kernel assumes bv == 0 (spec fill=zeros)"
    Ct0, St0 = _rope_tables(ypos, rope_freq)
    Ct1, St1 = _rope_tables(xpos, rope_freq)
    waxis = [_build_weights(Wq, Wk, Wv, Wo, a) for a in range(2)]
    in_maps = []
    for core in range(8):
        b, axis, half = core // 4, (core // 2) % 2, core % 2
        wq, wk, wv, wo = waxis[axis]
        if axis == 0:
            blkx = x[b, :, 32 * half : 32 * half + 32, :]     # [Y, 32, C]
            xT = np.transpose(blkx, (2, 1, 0)).reshape(C, NPOS)  # (c, o=x, t=y)
            Ct, St = Ct0, St0
        else:
            blkx = x[b, 32 * half : 32 * half + 32, :, :]     # [32, X, C]
            xT = np.transpose(blkx, (2, 0, 1)).reshape(C, NPOS)  # (c, o=y, t=x)
            Ct, St = Ct1, St1
        in_maps.append(
            dict(
                xT=np.ascontiguousarray(xT, np.float32),
                wq=wq, wk=wk, wv=wv, wo=wo,
                ctab=Ct.astype(ml_dtypes.bfloat16),
                stab=St.astype(ml_dtypes.bfloat16),
            )
        )
    return in_maps


def gather_output(results):
    out = np.zeros((B, Y, X, C), np.float32)
    for core in range(8):
        b, axis, half = core // 4, (core // 2) % 2, core % 2
        outT = results[core]["outT"].reshape(C, 32, 64)
        if axis == 0:
            out[b, :, 32 * half : 32 * half + 32, :] += np.transpose(outT, (2, 1, 0))
        else:
            out[b, 32 * half : 32 * half + 32, :, :] += np.transpose(outT, (1, 2, 0))
    return out


_CACHED = {}


def kernel(x, Wq, Wk, Wv, bv, Wo, rope_freq, ypos, xpos, mask):
    from concourse.bass_utils import run_bass_kernel_spmd

    x, Wq, Wk, Wv, bv, Wo, rope_freq, ypos, xpos = (
        np.asarray(a, np.float32) for a in (x, Wq, Wk, Wv, bv, Wo, rope_freq, ypos, xpos)
    )
    in_maps = prepare_in_maps(x, Wq, Wk, Wv, bv, Wo, rope_freq, ypos, xpos, mask)
    if "nc" not in _CACHED:
        _CACHED["nc"] = build_program()
    res = run_bass_kernel_spmd(_CACHED["nc"], in_maps, core_ids=list(range(8)))
    return gather_output(res.results)


# revision 34
# speedup vs baseline: 1.4293x; 1.0569x over previous
"""Axial sigmoid-attention Trainium2 kernel (8 NeuronCores, SPMD) — v5.

Sharding: core = b*4 + axis*2 + half; each core runs ONE axis over half the
non-attended spatial extent of one batch element (2048 positions = 32 outer
x 64 attended). Host gathers and sums the two axes' contributions.

RoPE decomposition (no cross-partition ops): logits accumulate in PSUM as
  A: (c_t q)·(c_s k) + (s_t q)·(s_s k)
  B: (c_t q)·(±s_s ksw) + (s_t q)·(∓c_s ksw),  ksw = x @ Wk_swapneg (host).

Schedule: per-block units are software-pipelined — attention(b) is emitted
interleaved with projections(b+1) and out-projection(b-1) so the PE always
has independent matmul work during sigmoid/drain latencies (keeps HAM warm).
PSUM rule respected throughout: matmuls from different row-groups never
write the same (bank, col-group).
"""

import numpy as np

B, Y, X, C = 2, 64, 64, 256
M, KG = 4, 2
H, HV = 32, 32
HH = H // 2
SCALE = 1.0 / np.sqrt(H)
DEN = 1.0 / np.sqrt(65.0)
NPOS = 2048
NBLK = 4
BLK = NPOS // NBLK



# ---------------------------------------------------------------- bass program
def build_program():
    import concourse.bacc as bacc
    import concourse.mybir as mybir
    from concourse.tile import TileContext

    dt = mybir.dt
    AF = mybir.ActivationFunctionType

    nc = bacc.Bacc()

    xT = nc.declare_dram_parameter("xT", [256, NPOS], dt.bfloat16, isOutput=False)
    wq = nc.declare_dram_parameter("wq", [256, 1024], dt.bfloat16, isOutput=False)
    wk = nc.declare_dram_parameter("wk", [256, 512], dt.bfloat16, isOutput=False)
    wv = nc.declare_dram_parameter("wv", [256, 256], dt.bfloat16, isOutput=False)
    wo = nc.declare_dram_parameter("wo", [128, 8 * 256], dt.bfloat16, isOutput=False)
    ctab = nc.declare_dram_parameter("ctab", [128, 2048], dt.bfloat16, isOutput=False)
    stab = nc.declare_dram_parameter("stab", [128, 2048], dt.bfloat16, isOutput=False)
    outT = nc.declare_dram_parameter("outT", [256, NPOS], dt.float32, isOutput=True)

    with TileContext(nc) as tc:
        with (
            tc.tile_pool(name="fixed", bufs=1) as fixed,
            tc.tile_pool(name="qblk", bufs=2) as qblk,
            tc.tile_pool(name="kblk", bufs=2) as kblk,
            tc.tile_pool(name="wblk", bufs=3) as wpool,
            tc.tile_pool(name="vcblk", bufs=2) as vcpool,
            tc.tile_pool(name="pproj", bufs=2, space="PSUM") as pproj,
            tc.tile_pool(name="pqk", bufs=3, space="PSUM") as pqk,
        ):
            xt_sb = fixed.tile([128, 2, NPOS], dt.bfloat16, tag="xt")
            wq_sb = fixed.tile([128, 2, 1024], dt.bfloat16, tag="wq")
            wk_sb = fixed.tile([128, 2, 512], dt.bfloat16, tag="wk")
            wv_sb = fixed.tile([128, 2, 256], dt.bfloat16, tag="wv")
            wo_sb = fixed.tile([128, 8, 256], dt.bfloat16, tag="wo")
            ct_sb = fixed.tile([128, 2048], dt.bfloat16, tag="ct")
            st_sb = fixed.tile([128, 2048], dt.bfloat16, tag="st")
            # weights first, then x chunked per block so block-0 proj starts early
            for cc in range(2):
                nc.sync.dma_start(out=wq_sb[:, cc, 0:512], in_=wq[128 * cc : 128 * cc + 128, 0:512])
                nc.sync.dma_start(
                    out=xt_sb[:, cc, 0:BLK],
                    in_=xT[128 * cc : 128 * cc + 128, 0:BLK],
                )
            for cc in range(2):
                nc.sync.dma_start(out=wq_sb[:, cc, 512:1024], in_=wq[128 * cc : 128 * cc + 128, 512:1024])
                nc.gpsimd.dma_start(out=wk_sb[:, cc], in_=wk[128 * cc : 128 * cc + 128])
                nc.gpsimd.dma_start(out=wv_sb[:, cc], in_=wv[128 * cc : 128 * cc + 128])
            nc.gpsimd.dma_start(out=ct_sb[:], in_=ctab[:])
            nc.gpsimd.dma_start(out=st_sb[:], in_=stab[:])
            nc.gpsimd.dma_start(out=wo_sb[:], in_=wo[:].rearrange("p (c n) -> p c n", c=8))
            for b in range(1, NBLK):
                for cc in range(2):
                    nc.scalar.dma_start(
                        out=xt_sb[:, cc, b * BLK : (b + 1) * BLK],
                        in_=xT[128 * cc : 128 * cc + 128, b * BLK : (b + 1) * BLK],
                    )

            # vector-clock warmups: each engine observes every input-DMA lane
            # via ops that depend on exactly one DMA (HW allows one sync wait
            # per instruction).
            warm = pqk.tile([128, 512], dt.float32, tag="qk2", name="warm")
            _wi = [0]

            def warm_touch(sl):
                i = _wi[0]; _wi[0] += 1
                nc.tensor.matmul(warm[0:8, 8 * i : 8 * i + 8], lhsT=sl, rhs=sl,
                                 start=True, stop=True)

            # stage 1: only what the first q-units need
            for sl in [wq_sb[:, 0, 0:8], wq_sb[:, 1, 0:8],
                       xt_sb[:, 0, 0:8], xt_sb[:, 1, 0:8]]:
                warm_touch(sl)
            wscr = fixed.tile([128, 32], dt.bfloat16, tag="wscr")
            nc.vector.tensor_copy(wscr[:, 0:8], ct_sb[:, 0:8])
            nc.vector.tensor_copy(wscr[:, 8:16], st_sb[:, 0:8])
            nc.gpsimd.tensor_copy(wscr[:, 16:24], ct_sb[:, 0:8])
            nc.gpsimd.tensor_copy(wscr[:, 24:32], st_sb[:, 0:8])

            S = {}  # per-block live tiles

            def proj_units(blk):
                """Generator of emit-callbacks for block `blk` projections."""
                p0 = blk * BLK
                st = S[blk] = {}
                st["q"] = [qblk.tile([128, M * BLK], dt.bfloat16, tag=f"q{c}", name=f"q{c}_{blk}") for c in range(2)]
                st["cq"] = [qblk.tile([128, M * BLK], dt.bfloat16, tag=f"cq{c}", name=f"cq{c}_{blk}") for c in range(2)]
                st["sq"] = [qblk.tile([128, M * BLK], dt.bfloat16, tag=f"sq{c}", name=f"sq{c}_{blk}") for c in range(2)]
                st["ke"] = [kblk.tile([128, BLK], dt.bfloat16, tag=f"ke{c}", name=f"ke{c}_{blk}") for c in range(4)]
                st["rk"] = [kblk.tile([128, BLK], dt.bfloat16, tag=f"rk{c}", name=f"rk{c}_{blk}") for c in range(2)]
                st["rpk"] = [kblk.tile([128, BLK], dt.bfloat16, tag=f"rpk{c}", name=f"rpk{c}_{blk}") for c in range(2)]
                st["vt"] = [kblk.tile([128, 256], dt.bfloat16, tag=f"vt{i}", name=f"vt{i}_{blk}") for i in range(4)]
                st["vc"] = [vcpool.tile([128, 8 * 256], dt.bfloat16, tag=f"vc{q}", name=f"vc{q}_{blk}") for q in range(2)]

                def q_unit(m, cht):
                    def emit():
                        ps = pproj.tile([128, BLK], dt.float32, tag="proj", name=f"qp{m}{cht}_{blk}")
                        for cc in range(2):
                            nc.tensor.matmul(
                                ps[:],
                                lhsT=wq_sb[:, cc, m * 256 + cht * 128 : m * 256 + (cht + 1) * 128],
                                rhs=xt_sb[:, cc, p0 : p0 + BLK],
                                start=(cc == 0), stop=(cc == 1),
                            )
                        if cht == 0:
                            nc.vector.tensor_copy(st["q"][cht][:, m * BLK : (m + 1) * BLK], ps[:])
                        else:
                            nc.scalar.copy(st["q"][cht][:, m * BLK : (m + 1) * BLK], ps[:])
                    return emit

                def k_unit(cht):
                    def emit():
                        ps = pproj.tile([128, BLK], dt.float32, tag="proj", name=f"kp{cht}_{blk}")
                        for cc in range(2):
                            nc.tensor.matmul(
                                ps[:],
                                lhsT=wk_sb[:, cc, cht * 128 : (cht + 1) * 128],
                                rhs=xt_sb[:, cc, p0 : p0 + BLK],
                                start=(cc == 0), stop=(cc == 1),
                            )
                        nc.scalar.copy(st["ke"][cht][:], ps[:])
                    return emit

                def vt_unit(op2):
                    def emit():
                        pp = p0 + op2 * 128
                        ps = pproj.tile([128, BLK], dt.float32, tag="proj", name=f"vtp{op2}_{blk}")
                        for cc in range(2):
                            nc.tensor.matmul(
                                ps[:, :256],
                                lhsT=xt_sb[:, cc, pp : pp + 128],
                                rhs=wv_sb[:, cc],
                                start=(cc == 0), stop=(cc == 1),
                            )
                        nc.scalar.copy(st["vt"][op2][:], ps[:, :256])
                    return emit

                def rope_unit(cht):
                    def emit():
                        nc.vector.tensor_mul(st["cq"][cht][:], st["q"][cht][:], ct_sb[:])
                        nc.vector.tensor_mul(st["sq"][cht][:], st["q"][cht][:], st_sb[:])
                    return emit

                def kprod_unit(dh):
                    def emit():
                        pk = kblk.tile([128, BLK], dt.bfloat16, tag="pk", name=f"pk{dh}_{blk}")
                        psw = kblk.tile([128, BLK], dt.bfloat16, tag="psw", name=f"psw{dh}_{blk}")
                        nc.vector.tensor_mul(pk[:], st["ke"][dh][:], ct_sb[:, :BLK])
                        nc.vector.tensor_mul(psw[:], st["ke"][2 + dh][:], st_sb[:, :BLK])
                        if dh == 0:
                            nc.vector.tensor_add(st["rk"][dh][:], pk[:], psw[:])
                        else:
                            nc.vector.tensor_sub(st["rk"][dh][:], pk[:], psw[:])
                        pk2 = kblk.tile([128, BLK], dt.bfloat16, tag="pk2", name=f"pk2{dh}_{blk}")
                        psw2 = kblk.tile([128, BLK], dt.bfloat16, tag="psw2", name=f"psw2{dh}_{blk}")
                        nc.vector.tensor_mul(pk2[:], st["ke"][dh][:], st_sb[:, :BLK])
                        nc.vector.tensor_mul(psw2[:], st["ke"][2 + dh][:], ct_sb[:, :BLK])
                        if dh == 0:
                            nc.vector.tensor_sub(st["rpk"][dh][:], pk2[:], psw2[:])
                        else:
                            nc.vector.tensor_add(st["rpk"][dh][:], pk2[:], psw2[:])
                    return emit

                units = [q_unit(m, cht) for m in range(M) for cht in range(2)]
                units.append(rope_unit(0))
                units.append(rope_unit(1))
                units += [k_unit(c) for c in range(4)]
                units.append(kprod_unit(0))
                units.append(kprod_unit(1))
                units += [vt_unit(i) for i in range(4)]
                return units

            def attn_units(blk):
                st = S[blk]
                units = []

                def qk_unit(op2p, quad):
                    def emit():
                        qkA = pqk.tile([128, 1024], dt.float32, tag="qk2", name=f"qkA{op2p}_{quad}_{blk}")
                        qkB = pqk.tile([128, 1024], dt.float32, tag="qk2", name=f"qkB{op2p}_{quad}_{blk}")
                        qkg = [qkA, qkA, qkB, qkB]
                        for g4 in range(4):
                            for op2l in range(2):
                                for oo in range(2):
                                    o = (op2p * 2 + op2l) * 2 + oo
                                    for ph, (kt, ut) in enumerate(((st["rk"], st["cq"]), (st["rpk"], st["sq"]))):
                                        foff = 512 * (g4 % 2) + 256 * op2l
                                        nc.tensor.matmul(
                                            qkg[g4][64 * oo : 64 * oo + 64, foff : foff + 256],
                                            lhsT=kt[quad][32 * g4 : 32 * g4 + 32, o * 64 : (o + 1) * 64],
                                            rhs=ut[quad][:]
                                            .rearrange("p (m t) -> p m t", m=M)[
                                                32 * g4 : 32 * g4 + 32, :, o * 64 : (o + 1) * 64
                                            ],
                                            start=(ph == 0), stop=(ph == 1),
                                            tile_position=(32 * g4, 64 * oo),
                                        )
                        w_sb = wpool.tile([128, 2048], dt.bfloat16, tag="w", name=f"w{op2p}_{quad}_{blk}")
                        for bk in range(2):
                            nc.scalar.activation(w_sb[:, 512 * bk : 512 * bk + 512],
                                                 qkA[:, 512 * bk : 512 * bk + 512], AF.Sigmoid, scale=SCALE)
                            nc.scalar.activation(w_sb[:, 1024 + 512 * bk : 1536 + 512 * bk],
                                                 qkB[:, 512 * bk : 512 * bk + 512], AF.Sigmoid, scale=SCALE)
                        st[f"w{op2p}_{quad}"] = w_sb
                    return emit

                def av_unit(op2p, op2l):
                    def emit():
                        op2 = op2p * 2 + op2l
                        av = pqk.tile([128, 1024], dt.float32, tag="qk2", name=f"av{op2}_{blk}")
                        for quad in range(2):
                            for oo in range(2):
                                for g4 in range(4):
                                    woff = 1024 * (g4 // 2) + 512 * (g4 % 2) + 256 * op2l
                                    nc.tensor.matmul(
                                        av[32 * g4 : 32 * g4 + 32,
                                           512 * oo + 256 * quad : 512 * oo + 256 * quad + 256],
                                        lhsT=st["vt"][op2][
                                            64 * oo : 64 * oo + 64,
                                            32 * (quad * 4 + g4) : 32 * (quad * 4 + g4) + 32,
                                        ],
                                        rhs=st[f"w{op2p}_{quad}"][64 * oo : 64 * oo + 64, woff : woff + 256],
                                        start=True, stop=True,
                                        tile_position=(64 * oo, 32 * g4),
                                    )
                        av4 = av[:].rearrange("p (oo q mt) -> p oo q mt", oo=2, q=2)
                        for quad in range(2):
                            nc.vector.tensor_copy(
                                st["vc"][quad][:, op2 * 512 : (op2 + 1) * 512]
                                .rearrange("p (oo mt) -> p oo mt", oo=2),
                                av4[:, :, quad, :],
                            )
                    return emit

                for op2p in range(2):
                    units.append(qk_unit(op2p, 0))
                    units.append(qk_unit(op2p, 1))
                    units.append(av_unit(op2p, 0))
                    units.append(av_unit(op2p, 1))
                return units

            def outproj_units(blk):
                st = S[blk]

                def unit(och):
                    def emit():
                        ps = pproj.tile([128, BLK], dt.float32, tag="proj", name=f"ops{och}_{blk}")
                        for ch in range(8):
                            m, quad = ch // 2, ch % 2
                            nc.tensor.matmul(
                                ps[:],
                                lhsT=wo_sb[:, ch, och * 128 : (och + 1) * 128],
                                rhs=st["vc"][quad][:]
                                .rearrange("p (o m t) -> p o m t", o=8, m=M)[:, :, m, :],
                                start=(ch == 0), stop=(ch == 7),
                            )
                        o_sb = wpool.tile([128, BLK], dt.float32, tag="osb", name=f"osb{och}_{blk}")
                        nc.vector.tensor_copy(o_sb[:], ps[:])
                        nc.sync.dma_start(
                            out=outT[och * 128 : (och + 1) * 128, blk * BLK : (blk + 1) * BLK],
                            in_=o_sb[:],
                        )
                    return emit

                return [unit(0), unit(1)]

            def interleave(primary, fillers):
                """Emit primary units with filler units spread between them."""
                if not primary:
                    for f in fillers:
                        f()
                    return
                k = len(fillers)
                n = len(primary)
                fi = 0
                for i, p in enumerate(primary):
                    p()
                    take = (k * (i + 1)) // n - fi
                    for _ in range(take):
                        fillers[fi]()
                        fi += 1

            # prologue: block-0 projections, warming late tensors just in time
            p0units = proj_units(0)
            # unit order: 8 q-units, 2 rope, 4 k-units, 2 kprod, 4 vt-units
            for i, u in enumerate(p0units):
                if i == 4:
                    warm_touch(wq_sb[:, 0, 512:520]); warm_touch(wq_sb[:, 1, 512:520])
                if i == 10:
                    warm_touch(wk_sb[:, 0, 0:8]); warm_touch(wk_sb[:, 1, 0:8])
                if i == 16:
                    warm_touch(wv_sb[:, 0, 0:8]); warm_touch(wv_sb[:, 1, 0:8])
                u()
            warm_touch(wo_sb[:, 0, 0:8])
            warm_touch(ct_sb[:, 0:8]); warm_touch(st_sb[:, 0:8])
            for blk in range(NBLK):
                fillers = []
                if blk + 1 < NBLK:
                    for cc in range(2):
                        warm_touch(xt_sb[:, cc, (blk + 1) * BLK : (blk + 1) * BLK + 8])
                    fillers += proj_units(blk + 1)
                if blk - 1 >= 0:
                    fillers += outproj_units(blk - 1)
                interleave(attn_units(blk), fillers)
            for u in outproj_units(NBLK - 1):
                u()

    nc.compile()
    return nc


# ---------------------------------------------------------------- host side
def _rope_tables(pos_arr, rope_freq):
    scaling = np.pi / np.stack([np.linspace(1, 30, HH), np.linspace(0.1, 1, HH)], -1)
    freq = rope_freq * scaling.astype(np.float32)
    phi = (pos_arr[:, None, :] * freq[None, :, :]).sum(-1)   # [64, HH]
    cs, sn = np.cos(phi), np.sin(phi)
    Ct = np.repeat(cs.T, 2, axis=0).astype(np.float32)        # [32, 64] rows h
    St = np.repeat(sn.T, 2, axis=0).astype(np.float32)
    return np.tile(Ct, (4, 32)), np.tile(St, (4, 32))         # [128, 2048]


def _build_weights(Wq, Wk, Wv, Wo, axis):
    Wq_a = Wq[:, 2 * axis : 2 * axis + 2]                     # [C, d, v, M, KG, H]
    wq = np.transpose(Wq_a, (0, 3, 1, 2, 4, 5)).reshape(C, 1024)  # (m, d, v, kg, h)
    Wk_a = Wk[:, 2 * axis : 2 * axis + 2].reshape(C, 2, 2, KG, HH, 2)
    Wk_swap = np.stack([Wk_a[..., 1], -Wk_a[..., 0]], -1)
    wk = np.concatenate([Wk_a.reshape(C, 256), Wk_swap.reshape(C, 256)], 1)  # (e,d,v,kg,h)
    wv = (Wv[:, 2 * axis : 2 * axis + 2].reshape(C, 256) * DEN).astype(np.float32)
    Wo_a = Wo[2 * axis : 2 * axis + 2]                        # [d, v, M, KG, HV, C]
    Wo_perm = np.transpose(Wo_a, (2, 0, 1, 3, 4, 5)).reshape(M * 256, C)  # (m, c, f)
    wo = Wo_perm.reshape(8, 128, 256).transpose(1, 0, 2).reshape(128, 8 * 256)
    import ml_dtypes
    return (np.ascontiguousarray(wq).astype(ml_dtypes.bfloat16),
            np.ascontiguousarray(wk).astype(ml_dtypes.bfloat16),
            np.ascontiguousarray(wv).astype(ml_dtypes.bfloat16),
            np.ascontiguousarray(wo).astype(ml_dtypes.bfloat16))


def prepare_in_maps(x, Wq, Wk, Wv, bv, Wo, rope_freq, ypos, xpos, mask):
    import ml_dtypes

    assert np.abs(bv).max() == 0.0, "kernel assumes bv == 0 (spec fill=zeros)"
    Ct0, St0 = _rope_tables(ypos, rope_freq)
    Ct1, St1 = _rope_tables(xpos, rope_freq)
    waxis = [_build_weights(Wq, Wk, Wv, Wo, a) for a in range(2)]
    in_maps = []
    for core in range(8):
        b, axis, half = core // 4, (core // 2) % 2, core % 2
        wq, wk, wv, wo = waxis[axis]
        if axis == 0:
            blkx = x[b, :, 32 * half : 32 * half + 32, :]     # [Y, 32, C]
            xT = np.transpose(blkx, (2, 1, 0)).reshape(C, NPOS)  # (c, o=x, t=y)
            Ct, St = Ct0, St0
        else:
            blkx = x[b, 32 * half : 32 * half + 32, :, :]     # [32, X, C]
            xT = np.transpose(blkx, (2, 0, 1)).reshape(C, NPOS)  # (c, o=y, t=x)
            Ct, St = Ct1, St1
        in_maps.append(
            dict(
                xT=np.ascontiguousarray(xT).astype(ml_dtypes.bfloat16),
                wq=wq, wk=wk, wv=wv, wo=wo,
                ctab=Ct.astype(ml_dtypes.bfloat16),
                stab=St.astype(ml_dtypes.bfloat16),
            )
        )
    return in_maps


def gather_output(results):
    out = np.zeros((B, Y, X, C), np.float32)
    for core in range(8):
        b, axis, half = core // 4, (core // 2) % 2, core % 2
        outT = results[core]["outT"].reshape(C, 32, 64)
        if axis == 0:
            out[b, :, 32 * half : 32 * half + 32, :] += np.transpose(outT, (2, 1, 0))
        else:
            out[b, 32 * half : 32 * half + 32, :, :] += np.transpose(outT, (1, 2, 0))
    return out


_CACHED = {}


def kernel(x, Wq, Wk, Wv, bv, Wo, rope_freq, ypos, xpos, mask):
    from concourse.bass_utils import run_bass_kernel_spmd

    x, Wq, Wk, Wv, bv, Wo, rope_freq, ypos, xpos = (
        np.asarray(a, np.float32) for a in (x, Wq, Wk, Wv, bv, Wo, rope_freq, ypos, xpos)
    )
    in_maps = prepare_in_maps(x, Wq, Wk, Wv, bv, Wo, rope_freq, ypos, xpos, mask)
    if "nc" not in _CACHED:
        _CACHED["nc"] = build_program()
    res = run_bass_kernel_spmd(_CACHED["nc"], in_maps, core_ids=list(range(8)))
    return gather_output(res.results)


# revision 35
# speedup vs baseline: 1.4798x; 1.0353x over previous
"""Axial sigmoid-attention Trainium2 kernel (8 NeuronCores, SPMD) — v5.

Sharding: core = b*4 + axis*2 + half; each core runs ONE axis over half the
non-attended spatial extent of one batch element (2048 positions = 32 outer
x 64 attended). Host gathers and sums the two axes' contributions.

RoPE decomposition (no cross-partition ops): logits accumulate in PSUM as
  A: (c_t q)·(c_s k) + (s_t q)·(s_s k)
  B: (c_t q)·(±s_s ksw) + (s_t q)·(∓c_s ksw),  ksw = x @ Wk_swapneg (host).

Schedule: per-block units are software-pipelined — attention(b) is emitted
interleaved with projections(b+1) and out-projection(b-1) so the PE always
has independent matmul work during sigmoid/drain latencies (keeps HAM warm).
PSUM rule respected throughout: matmuls from different row-groups never
write the same (bank, col-group).
"""

import numpy as np

B, Y, X, C = 2, 64, 64, 256
M, KG = 4, 2
H, HV = 32, 32
HH = H // 2
SCALE = 1.0 / np.sqrt(H)
DEN = 1.0 / np.sqrt(65.0)
NPOS = 2048
NBLK = 4
BLK = NPOS // NBLK



# ---------------------------------------------------------------- bass program
def build_program():
    import concourse.bacc as bacc
    import concourse.mybir as mybir
    from concourse.tile import TileContext

    dt = mybir.dt
    AF = mybir.ActivationFunctionType

    nc = bacc.Bacc()

    xT = nc.declare_dram_parameter("xT", [256, NPOS], dt.bfloat16, isOutput=False)
    wq = nc.declare_dram_parameter("wq", [256, 1024], dt.bfloat16, isOutput=False)
    wk = nc.declare_dram_parameter("wk", [256, 512], dt.bfloat16, isOutput=False)
    wv = nc.declare_dram_parameter("wv", [256, 256], dt.bfloat16, isOutput=False)
    wo = nc.declare_dram_parameter("wo", [128, 8 * 256], dt.bfloat16, isOutput=False)
    ctab = nc.declare_dram_parameter("ctab", [128, 2048], dt.bfloat16, isOutput=False)
    stab = nc.declare_dram_parameter("stab", [128, 2048], dt.bfloat16, isOutput=False)
    outT = nc.declare_dram_parameter("outT", [256, NPOS], dt.float32, isOutput=True)

    with TileContext(nc) as tc:
        with (
            tc.tile_pool(name="fixed", bufs=1) as fixed,
            tc.tile_pool(name="qblk", bufs=2) as qblk,
            tc.tile_pool(name="kblk", bufs=2) as kblk,
            tc.tile_pool(name="wblk", bufs=3) as wpool,
            tc.tile_pool(name="vcblk", bufs=2) as vcpool,
            tc.tile_pool(name="pproj", bufs=2, space="PSUM") as pproj,
            tc.tile_pool(name="pqk", bufs=3, space="PSUM") as pqk,
        ):
            xt_sb = fixed.tile([128, 2, NPOS], dt.bfloat16, tag="xt")
            wq_sb = fixed.tile([128, 2, 1024], dt.bfloat16, tag="wq")
            wk_sb = fixed.tile([128, 2, 512], dt.bfloat16, tag="wk")
            wv_sb = fixed.tile([128, 2, 256], dt.bfloat16, tag="wv")
            wo_sb = fixed.tile([128, 8, 256], dt.bfloat16, tag="wo")
            ct_sb = fixed.tile([128, 2048], dt.bfloat16, tag="ct")
            st_sb = fixed.tile([128, 2048], dt.bfloat16, tag="st")
            # weights first, then x chunked per block so block-0 proj starts early
            for cc in range(2):
                nc.sync.dma_start(out=wq_sb[:, cc, 0:512], in_=wq[128 * cc : 128 * cc + 128, 0:512])
                nc.sync.dma_start(
                    out=xt_sb[:, cc, 0:BLK],
                    in_=xT[128 * cc : 128 * cc + 128, 0:BLK],
                )
            for cc in range(2):
                nc.sync.dma_start(out=wq_sb[:, cc, 512:1024], in_=wq[128 * cc : 128 * cc + 128, 512:1024])
                nc.scalar.dma_start(out=wk_sb[:, cc], in_=wk[128 * cc : 128 * cc + 128])
                nc.scalar.dma_start(out=wv_sb[:, cc], in_=wv[128 * cc : 128 * cc + 128])
            nc.scalar.dma_start(out=ct_sb[:], in_=ctab[:])
            nc.scalar.dma_start(out=st_sb[:], in_=stab[:])
            nc.gpsimd.dma_start(out=wo_sb[:], in_=wo[:].rearrange("p (c n) -> p c n", c=8))
            for b in range(1, NBLK):
                for cc in range(2):
                    nc.scalar.dma_start(
                        out=xt_sb[:, cc, b * BLK : (b + 1) * BLK],
                        in_=xT[128 * cc : 128 * cc + 128, b * BLK : (b + 1) * BLK],
                    )

            # vector-clock warmups: each engine observes every input-DMA lane
            # via ops that depend on exactly one DMA (HW allows one sync wait
            # per instruction).
            warm = pqk.tile([128, 512], dt.float32, tag="qk2", name="warm")
            _wi = [0]

            def warm_touch(sl):
                i = _wi[0]; _wi[0] += 1
                nc.tensor.matmul(warm[0:8, 8 * i : 8 * i + 8], lhsT=sl, rhs=sl,
                                 start=True, stop=True)

            # stage 1: only what the first q-units need
            for sl in [wq_sb[:, 0, 0:8], wq_sb[:, 1, 0:8],
                       xt_sb[:, 0, 0:8], xt_sb[:, 1, 0:8]]:
                warm_touch(sl)
            wscr = fixed.tile([128, 32], dt.bfloat16, tag="wscr")
            nc.vector.tensor_copy(wscr[:, 0:8], ct_sb[:, 0:8])
            nc.vector.tensor_copy(wscr[:, 8:16], st_sb[:, 0:8])
            nc.gpsimd.tensor_copy(wscr[:, 16:24], ct_sb[:, 0:8])
            nc.gpsimd.tensor_copy(wscr[:, 24:32], st_sb[:, 0:8])

            S = {}  # per-block live tiles

            def proj_units(blk):
                """Generator of emit-callbacks for block `blk` projections."""
                p0 = blk * BLK
                st = S[blk] = {}
                st["q"] = [qblk.tile([128, M * BLK], dt.bfloat16, tag=f"q{c}", name=f"q{c}_{blk}") for c in range(2)]
                st["cq"] = [qblk.tile([128, M * BLK], dt.bfloat16, tag=f"cq{c}", name=f"cq{c}_{blk}") for c in range(2)]
                st["sq"] = [qblk.tile([128, M * BLK], dt.bfloat16, tag=f"sq{c}", name=f"sq{c}_{blk}") for c in range(2)]
                st["ke"] = [kblk.tile([128, BLK], dt.bfloat16, tag=f"ke{c}", name=f"ke{c}_{blk}") for c in range(4)]
                st["rk"] = [kblk.tile([128, BLK], dt.bfloat16, tag=f"rk{c}", name=f"rk{c}_{blk}") for c in range(2)]
                st["rpk"] = [kblk.tile([128, BLK], dt.bfloat16, tag=f"rpk{c}", name=f"rpk{c}_{blk}") for c in range(2)]
                st["vt"] = [kblk.tile([128, 256], dt.bfloat16, tag=f"vt{i}", name=f"vt{i}_{blk}") for i in range(4)]
                st["vc"] = [vcpool.tile([128, 8 * 256], dt.bfloat16, tag=f"vc{q}", name=f"vc{q}_{blk}") for q in range(2)]

                def q_unit(m, cht):
                    def emit():
                        ps = pproj.tile([128, BLK], dt.float32, tag="proj", name=f"qp{m}{cht}_{blk}")
                        for cc in range(2):
                            nc.tensor.matmul(
                                ps[:],
                                lhsT=wq_sb[:, cc, m * 256 + cht * 128 : m * 256 + (cht + 1) * 128],
                                rhs=xt_sb[:, cc, p0 : p0 + BLK],
                                start=(cc == 0), stop=(cc == 1),
                            )
                        if cht == 0:
                            nc.vector.tensor_copy(st["q"][cht][:, m * BLK : (m + 1) * BLK], ps[:])
                        else:
                            nc.scalar.copy(st["q"][cht][:, m * BLK : (m + 1) * BLK], ps[:])
                    return emit

                def k_unit(cht):
                    def emit():
                        ps = pproj.tile([128, BLK], dt.float32, tag="proj", name=f"kp{cht}_{blk}")
                        for cc in range(2):
                            nc.tensor.matmul(
                                ps[:],
                                lhsT=wk_sb[:, cc, cht * 128 : (cht + 1) * 128],
                                rhs=xt_sb[:, cc, p0 : p0 + BLK],
                                start=(cc == 0), stop=(cc == 1),
                            )
                        nc.scalar.copy(st["ke"][cht][:], ps[:])
                    return emit

                def vt_unit(op2):
                    def emit():
                        pp = p0 + op2 * 128
                        ps = pproj.tile([128, BLK], dt.float32, tag="proj", name=f"vtp{op2}_{blk}")
                        for cc in range(2):
                            nc.tensor.matmul(
                                ps[:, :256],
                                lhsT=xt_sb[:, cc, pp : pp + 128],
                                rhs=wv_sb[:, cc],
                                start=(cc == 0), stop=(cc == 1),
                            )
                        nc.scalar.copy(st["vt"][op2][:], ps[:, :256])
                    return emit

                def rope_unit(cht):
                    def emit():
                        nc.vector.tensor_mul(st["cq"][cht][:], st["q"][cht][:], ct_sb[:])
                        nc.vector.tensor_mul(st["sq"][cht][:], st["q"][cht][:], st_sb[:])
                    return emit

                def kprod_unit(dh):
                    def emit():
                        pk = kblk.tile([128, BLK], dt.bfloat16, tag="pk", name=f"pk{dh}_{blk}")
                        psw = kblk.tile([128, BLK], dt.bfloat16, tag="psw", name=f"psw{dh}_{blk}")
                        nc.vector.tensor_mul(pk[:], st["ke"][dh][:], ct_sb[:, :BLK])
                        nc.vector.tensor_mul(psw[:], st["ke"][2 + dh][:], st_sb[:, :BLK])
                        if dh == 0:
                            nc.vector.tensor_add(st["rk"][dh][:], pk[:], psw[:])
                        else:
                            nc.vector.tensor_sub(st["rk"][dh][:], pk[:], psw[:])
                        pk2 = kblk.tile([128, BLK], dt.bfloat16, tag="pk2", name=f"pk2{dh}_{blk}")
                        psw2 = kblk.tile([128, BLK], dt.bfloat16, tag="psw2", name=f"psw2{dh}_{blk}")
                        nc.vector.tensor_mul(pk2[:], st["ke"][dh][:], st_sb[:, :BLK])
                        nc.vector.tensor_mul(psw2[:], st["ke"][2 + dh][:], ct_sb[:, :BLK])
                        if dh == 0:
                            nc.vector.tensor_sub(st["rpk"][dh][:], pk2[:], psw2[:])
                        else:
                            nc.vector.tensor_add(st["rpk"][dh][:], pk2[:], psw2[:])
                    return emit

                units = [q_unit(m, cht) for m in range(M) for cht in range(2)]
                units.append(rope_unit(0))
                units.append(rope_unit(1))
                units += [k_unit(c) for c in range(4)]
                units.append(kprod_unit(0))
                units.append(kprod_unit(1))
                units += [vt_unit(i) for i in range(4)]
                return units

            def attn_units(blk):
                st = S[blk]
                units = []

                def qk_unit(op2p, quad):
                    def emit():
                        qkA = pqk.tile([128, 1024], dt.float32, tag="qk2", name=f"qkA{op2p}_{quad}_{blk}")
                        qkB = pqk.tile([128, 1024], dt.float32, tag="qk2", name=f"qkB{op2p}_{quad}_{blk}")
                        qkg = [qkA, qkA, qkB, qkB]
                        for g4 in range(4):
                            for op2l in range(2):
                                for oo in range(2):
                                    o = (op2p * 2 + op2l) * 2 + oo
                                    for ph, (kt, ut) in enumerate(((st["rk"], st["cq"]), (st["rpk"], st["sq"]))):
                                        foff = 512 * (g4 % 2) + 256 * op2l
                                        nc.tensor.matmul(
                                            qkg[g4][64 * oo : 64 * oo + 64, foff : foff + 256],
                                            lhsT=kt[quad][32 * g4 : 32 * g4 + 32, o * 64 : (o + 1) * 64],
                                            rhs=ut[quad][:]
                                            .rearrange("p (m t) -> p m t", m=M)[
                                                32 * g4 : 32 * g4 + 32, :, o * 64 : (o + 1) * 64
                                            ],
                                            start=(ph == 0), stop=(ph == 1),
                                            tile_position=(32 * g4, 64 * oo),
                                        )
                        w_sb = wpool.tile([128, 2048], dt.bfloat16, tag="w", name=f"w{op2p}_{quad}_{blk}")
                        for bk in range(2):
                            nc.scalar.activation(w_sb[:, 512 * bk : 512 * bk + 512],
                                                 qkA[:, 512 * bk : 512 * bk + 512], AF.Sigmoid, scale=SCALE)
                            nc.scalar.activation(w_sb[:, 1024 + 512 * bk : 1536 + 512 * bk],
                                                 qkB[:, 512 * bk : 512 * bk + 512], AF.Sigmoid, scale=SCALE)
                        st[f"w{op2p}_{quad}"] = w_sb
                    return emit

                def av_unit(op2p, op2l):
                    def emit():
                        op2 = op2p * 2 + op2l
                        av = pqk.tile([128, 1024], dt.float32, tag="qk2", name=f"av{op2}_{blk}")
                        for quad in range(2):
                            for oo in range(2):
                                for g4 in range(4):
                                    woff = 1024 * (g4 // 2) + 512 * (g4 % 2) + 256 * op2l
                                    nc.tensor.matmul(
                                        av[32 * g4 : 32 * g4 + 32,
                                           512 * oo + 256 * quad : 512 * oo + 256 * quad + 256],
                                        lhsT=st["vt"][op2][
                                            64 * oo : 64 * oo + 64,
                                            32 * (quad * 4 + g4) : 32 * (quad * 4 + g4) + 32,
                                        ],
                                        rhs=st[f"w{op2p}_{quad}"][64 * oo : 64 * oo + 64, woff : woff + 256],
                                        start=True, stop=True,
                                        tile_position=(64 * oo, 32 * g4),
                                    )
                        av4 = av[:].rearrange("p (oo q mt) -> p oo q mt", oo=2, q=2)
                        for quad in range(2):
                            nc.vector.tensor_copy(
                                st["vc"][quad][:, op2 * 512 : (op2 + 1) * 512]
                                .rearrange("p (oo mt) -> p oo mt", oo=2),
                                av4[:, :, quad, :],
                            )
                    return emit

                for op2p in range(2):
                    units.append(qk_unit(op2p, 0))
                    units.append(qk_unit(op2p, 1))
                    units.append(av_unit(op2p, 0))
                    units.append(av_unit(op2p, 1))
                return units

            def outproj_units(blk):
                st = S[blk]

                def unit(och):
                    def emit():
                        ps = pproj.tile([128, BLK], dt.float32, tag="proj", name=f"ops{och}_{blk}")
                        for ch in range(8):
                            m, quad = ch // 2, ch % 2
                            nc.tensor.matmul(
                                ps[:],
                                lhsT=wo_sb[:, ch, och * 128 : (och + 1) * 128],
                                rhs=st["vc"][quad][:]
                                .rearrange("p (o m t) -> p o m t", o=8, m=M)[:, :, m, :],
                                start=(ch == 0), stop=(ch == 7),
                            )
                        o_sb = wpool.tile([128, BLK], dt.float32, tag="osb", name=f"osb{och}_{blk}")
                        nc.vector.tensor_copy(o_sb[:], ps[:])
                        nc.sync.dma_start(
                            out=outT[och * 128 : (och + 1) * 128, blk * BLK : (blk + 1) * BLK],
                            in_=o_sb[:],
                        )
                    return emit

                return [unit(0), unit(1)]

            def interleave(primary, fillers):
                """Emit primary units with filler units spread between them."""
                if not primary:
                    for f in fillers:
                        f()
                    return
                k = len(fillers)
                n = len(primary)
                fi = 0
                for i, p in enumerate(primary):
                    p()
                    take = (k * (i + 1)) // n - fi
                    for _ in range(take):
                        fillers[fi]()
                        fi += 1

            # prologue: block-0 projections, warming late tensors just in time
            p0units = proj_units(0)
            # unit order: 8 q-units, 2 rope, 4 k-units, 2 kprod, 4 vt-units
            for i, u in enumerate(p0units):
                if i == 4:
                    warm_touch(wq_sb[:, 0, 512:520]); warm_touch(wq_sb[:, 1, 512:520])
                if i == 10:
                    warm_touch(wk_sb[:, 0, 0:8]); warm_touch(wk_sb[:, 1, 0:8])
                if i == 16:
                    warm_touch(wv_sb[:, 0, 0:8]); warm_touch(wv_sb[:, 1, 0:8])
                u()
            warm_touch(wo_sb[:, 0, 0:8])
            warm_touch(ct_sb[:, 0:8]); warm_touch(st_sb[:, 0:8])
            for blk in range(NBLK):
                fillers = []
                if blk + 1 < NBLK:
                    for cc in range(2):
                        warm_touch(xt_sb[:, cc, (blk + 1) * BLK : (blk + 1) * BLK + 8])
                    fillers += proj_units(blk + 1)
                if blk - 1 >= 0:
                    fillers += outproj_units(blk - 1)
                interleave(attn_units(blk), fillers)
            for u in outproj_units(NBLK - 1):
                u()

    nc.compile()
    return nc


# ---------------------------------------------------------------- host side
def _rope_tables(pos_arr, rope_freq):
    scaling = np.pi / np.stack([np.linspace(1, 30, HH), np.linspace(0.1, 1, HH)], -1)
    freq = rope_freq * scaling.astype(np.float32)
    phi = (pos_arr[:, None, :] * freq[None, :, :]).sum(-1)   # [64, HH]
    cs, sn = np.cos(phi), np.sin(phi)
    Ct = np.repeat(cs.T, 2, axis=0).astype(np.float32)        # [32, 64] rows h
    St = np.repeat(sn.T, 2, axis=0).astype(np.float32)
    return np.tile(Ct, (4, 32)), np.tile(St, (4, 32))         # [128, 2048]


def _build_weights(Wq, Wk, Wv, Wo, axis):
    Wq_a = Wq[:, 2 * axis : 2 * axis + 2]                     # [C, d, v, M, KG, H]
    wq = np.transpose(Wq_a, (0, 3, 1, 2, 4, 5)).reshape(C, 1024)  # (m, d, v, kg, h)
    Wk_a = Wk[:, 2 * axis : 2 * axis + 2].reshape(C, 2, 2, KG, HH, 2)
    Wk_swap = np.stack([Wk_a[..., 1], -Wk_a[..., 0]], -1)
    wk = np.concatenate([Wk_a.reshape(C, 256), Wk_swap.reshape(C, 256)], 1)  # (e,d,v,kg,h)
    wv = (Wv[:, 2 * axis : 2 * axis + 2].reshape(C, 256) * DEN).astype(np.float32)
    Wo_a = Wo[2 * axis : 2 * axis + 2]                        # [d, v, M, KG, HV, C]
    Wo_perm = np.transpose(Wo_a, (2, 0, 1, 3, 4, 5)).reshape(M * 256, C)  # (m, c, f)
    wo = Wo_perm.reshape(8, 128, 256).transpose(1, 0, 2).reshape(128, 8 * 256)
    import ml_dtypes
    return (np.ascontiguousarray(wq).astype(ml_dtypes.bfloat16),
            np.ascontiguousarray(wk).astype(ml_dtypes.bfloat16),
            np.ascontiguousarray(wv).astype(ml_dtypes.bfloat16),
            np.ascontiguousarray(wo).astype(ml_dtypes.bfloat16))


def prepare_in_maps(x, Wq, Wk, Wv, bv, Wo, rope_freq, ypos, xpos, mask):
    import ml_dtypes

    assert np.abs(bv).max() == 0.0, "kernel assumes bv == 0 (spec fill=zeros)"
    Ct0, St0 = _rope_tables(ypos, rope_freq)
    Ct1, St1 = _rope_tables(xpos, rope_freq)
    waxis = [_build_weights(Wq, Wk, Wv, Wo, a) for a in range(2)]
    in_maps = []
    for core in range(8):
        b, axis, half = core // 4, (core // 2) % 2, core % 2
        wq, wk, wv, wo = waxis[axis]
        if axis == 0:
            blkx = x[b, :, 32 * half : 32 * half + 32, :]     # [Y, 32, C]
            xT = np.transpose(blkx, (2, 1, 0)).reshape(C, NPOS)  # (c, o=x, t=y)
            Ct, St = Ct0, St0
        else:
            blkx = x[b, 32 * half : 32 * half + 32, :, :]     # [32, X, C]
            xT = np.transpose(blkx, (2, 0, 1)).reshape(C, NPOS)  # (c, o=y, t=x)
            Ct, St = Ct1, St1
        in_maps.append(
            dict(
                xT=np.ascontiguousarray(xT).astype(ml_dtypes.bfloat16),
                wq=wq, wk=wk, wv=wv, wo=wo,
                ctab=Ct.astype(ml_dtypes.bfloat16),
                stab=St.astype(ml_dtypes.bfloat16),
            )
        )
    return in_maps


def gather_output(results):
    out = np.zeros((B, Y, X, C), np.float32)
    for core in range(8):
        b, axis, half = core // 4, (core // 2) % 2, core % 2
        outT = results[core]["outT"].reshape(C, 32, 64)
        if axis == 0:
            out[b, :, 32 * half : 32 * half + 32, :] += np.transpose(outT, (2, 1, 0))
        else:
            out[b, 32 * half : 32 * half + 32, :, :] += np.transpose(outT, (1, 2, 0))
    return out


_CACHED = {}


def kernel(x, Wq, Wk, Wv, bv, Wo, rope_freq, ypos, xpos, mask):
    from concourse.bass_utils import run_bass_kernel_spmd

    x, Wq, Wk, Wv, bv, Wo, rope_freq, ypos, xpos = (
        np.asarray(a, np.float32) for a in (x, Wq, Wk, Wv, bv, Wo, rope_freq, ypos, xpos)
    )
    in_maps = prepare_in_maps(x, Wq, Wk, Wv, bv, Wo, rope_freq, ypos, xpos, mask)
    if "nc" not in _CACHED:
        _CACHED["nc"] = build_program()
    res = run_bass_kernel_spmd(_CACHED["nc"], in_maps, core_ids=list(range(8)))
    return gather_output(res.results)


# revision 36
# speedup vs baseline: 1.5413x; 1.0416x over previous
"""Axial sigmoid-attention Trainium2 kernel (8 NeuronCores, SPMD) — v5.

Sharding: core = b*4 + axis*2 + half; each core runs ONE axis over half the
non-attended spatial extent of one batch element (2048 positions = 32 outer
x 64 attended). Host gathers and sums the two axes' contributions.

RoPE decomposition (no cross-partition ops): logits accumulate in PSUM as
  A: (c_t q)·(c_s k) + (s_t q)·(s_s k)
  B: (c_t q)·(±s_s ksw) + (s_t q)·(∓c_s ksw),  ksw = x @ Wk_swapneg (host).

Schedule: per-block units are software-pipelined — attention(b) is emitted
interleaved with projections(b+1) and out-projection(b-1) so the PE always
has independent matmul work during sigmoid/drain latencies (keeps HAM warm).
PSUM rule respected throughout: matmuls from different row-groups never
write the same (bank, col-group).
"""

import numpy as np

B, Y, X, C = 2, 64, 64, 256
M, KG = 4, 2
H, HV = 32, 32
HH = H // 2
SCALE = 1.0 / np.sqrt(H)
DEN = 1.0 / np.sqrt(65.0)
NPOS = 2048
NBLK = 4
BLK = NPOS // NBLK



# ---------------------------------------------------------------- bass program
def build_program():
    import concourse.bacc as bacc
    import concourse.mybir as mybir
    from concourse.tile import TileContext

    dt = mybir.dt
    AF = mybir.ActivationFunctionType

    nc = bacc.Bacc()

    xT = nc.declare_dram_parameter("xT", [256, NPOS], dt.bfloat16, isOutput=False)
    wq = nc.declare_dram_parameter("wq", [256, 1024], dt.bfloat16, isOutput=False)
    wk = nc.declare_dram_parameter("wk", [256, 512], dt.bfloat16, isOutput=False)
    wv = nc.declare_dram_parameter("wv", [256, 256], dt.bfloat16, isOutput=False)
    wo = nc.declare_dram_parameter("wo", [128, 8 * 256], dt.bfloat16, isOutput=False)
    ctab = nc.declare_dram_parameter("ctab", [128, 2048], dt.bfloat16, isOutput=False)
    stab = nc.declare_dram_parameter("stab", [128, 2048], dt.bfloat16, isOutput=False)
    outT = nc.declare_dram_parameter("outT", [256, NPOS], dt.float32, isOutput=True)

    with TileContext(nc) as tc:
        with (
            tc.tile_pool(name="fixed", bufs=1) as fixed,
            tc.tile_pool(name="qblk", bufs=2) as qblk,
            tc.tile_pool(name="kblk", bufs=2) as kblk,
            tc.tile_pool(name="wblk", bufs=3) as wpool,
            tc.tile_pool(name="vcblk", bufs=2) as vcpool,
            tc.tile_pool(name="pproj", bufs=2, space="PSUM") as pproj,
            tc.tile_pool(name="pqk", bufs=3, space="PSUM") as pqk,
        ):
            xt_sb = fixed.tile([128, 2, NPOS], dt.bfloat16, tag="xt")
            wq_sb = fixed.tile([128, 2, 1024], dt.bfloat16, tag="wq")
            wk_sb = fixed.tile([128, 2, 512], dt.bfloat16, tag="wk")
            wv_sb = fixed.tile([128, 2, 256], dt.bfloat16, tag="wv")
            wo_sb = fixed.tile([128, 8, 256], dt.bfloat16, tag="wo")
            ct_sb = fixed.tile([128, 2048], dt.bfloat16, tag="ct")
            st_sb = fixed.tile([128, 2048], dt.bfloat16, tag="st")
            # weights first, then x chunked per block so block-0 proj starts early
            for cc in range(2):
                nc.sync.dma_start(out=wq_sb[:, cc, 0:512], in_=wq[128 * cc : 128 * cc + 128, 0:512])
                nc.sync.dma_start(
                    out=xt_sb[:, cc, 0:BLK],
                    in_=xT[128 * cc : 128 * cc + 128, 0:BLK],
                )
            for cc in range(2):
                nc.sync.dma_start(out=wq_sb[:, cc, 512:1024], in_=wq[128 * cc : 128 * cc + 128, 512:1024])
                nc.scalar.dma_start(out=wk_sb[:, cc], in_=wk[128 * cc : 128 * cc + 128])
                nc.scalar.dma_start(out=wv_sb[:, cc], in_=wv[128 * cc : 128 * cc + 128])
            nc.scalar.dma_start(out=ct_sb[:], in_=ctab[:])
            nc.scalar.dma_start(out=st_sb[:], in_=stab[:])
            nc.gpsimd.dma_start(out=wo_sb[:], in_=wo[:].rearrange("p (c n) -> p c n", c=8))
            for b in range(1, NBLK):
                for cc in range(2):
                    nc.scalar.dma_start(
                        out=xt_sb[:, cc, b * BLK : (b + 1) * BLK],
                        in_=xT[128 * cc : 128 * cc + 128, b * BLK : (b + 1) * BLK],
                    )

            # vector-clock warmups: each engine observes every input-DMA lane
            # via ops that depend on exactly one DMA (HW allows one sync wait
            # per instruction).
            warm = pqk.tile([128, 512], dt.float32, tag="qk2", name="warm")
            _wi = [0]

            def warm_touch(sl):
                i = _wi[0]; _wi[0] += 1
                nc.tensor.matmul(warm[0:8, 8 * i : 8 * i + 8], lhsT=sl, rhs=sl,
                                 start=True, stop=True)

            # stage 1: only what the first q-units need
            for sl in [wq_sb[:, 0, 0:8], wq_sb[:, 1, 0:8],
                       xt_sb[:, 0, 0:8], xt_sb[:, 1, 0:8]]:
                warm_touch(sl)
            wscr = fixed.tile([128, 32], dt.bfloat16, tag="wscr")
            nc.vector.tensor_copy(wscr[:, 0:8], ct_sb[:, 0:8])
            nc.vector.tensor_copy(wscr[:, 8:16], st_sb[:, 0:8])
            nc.gpsimd.tensor_copy(wscr[:, 16:24], ct_sb[:, 0:8])
            nc.gpsimd.tensor_copy(wscr[:, 24:32], st_sb[:, 0:8])

            S = {}  # per-block live tiles

            def proj_units(blk):
                """Generator of emit-callbacks for block `blk` projections."""
                p0 = blk * BLK
                st = S[blk] = {}
                st["q"] = [qblk.tile([128, M * BLK], dt.bfloat16, tag=f"q{c}", name=f"q{c}_{blk}") for c in range(2)]
                st["cq"] = [qblk.tile([128, M * BLK], dt.bfloat16, tag=f"cq{c}", name=f"cq{c}_{blk}") for c in range(2)]
                st["sq"] = [qblk.tile([128, M * BLK], dt.bfloat16, tag=f"sq{c}", name=f"sq{c}_{blk}") for c in range(2)]
                st["ke"] = [kblk.tile([128, BLK], dt.bfloat16, tag=f"ke{c}", name=f"ke{c}_{blk}") for c in range(4)]
                st["rk"] = [kblk.tile([128, BLK], dt.bfloat16, tag=f"rk{c}", name=f"rk{c}_{blk}") for c in range(2)]
                st["rpk"] = [kblk.tile([128, BLK], dt.bfloat16, tag=f"rpk{c}", name=f"rpk{c}_{blk}") for c in range(2)]
                st["vt"] = [kblk.tile([128, 256], dt.bfloat16, tag=f"vt{i}", name=f"vt{i}_{blk}") for i in range(4)]
                st["vc"] = [vcpool.tile([128, 8 * 256], dt.bfloat16, tag=f"vc{q}", name=f"vc{q}_{blk}") for q in range(2)]

                def q_unit(m, cht):
                    def emit():
                        ps = pproj.tile([128, BLK], dt.float32, tag="proj", name=f"qp{m}{cht}_{blk}")
                        for cc in range(2):
                            nc.tensor.matmul(
                                ps[:],
                                lhsT=wq_sb[:, cc, m * 256 + cht * 128 : m * 256 + (cht + 1) * 128],
                                rhs=xt_sb[:, cc, p0 : p0 + BLK],
                                start=(cc == 0), stop=(cc == 1),
                            )
                        if cht == 0:
                            nc.vector.tensor_copy(st["q"][cht][:, m * BLK : (m + 1) * BLK], ps[:])
                        else:
                            nc.scalar.copy(st["q"][cht][:, m * BLK : (m + 1) * BLK], ps[:])
                    return emit

                def k_unit(cht):
                    def emit():
                        ps = pproj.tile([128, BLK], dt.float32, tag="proj", name=f"kp{cht}_{blk}")
                        for cc in range(2):
                            nc.tensor.matmul(
                                ps[:],
                                lhsT=wk_sb[:, cc, cht * 128 : (cht + 1) * 128],
                                rhs=xt_sb[:, cc, p0 : p0 + BLK],
                                start=(cc == 0), stop=(cc == 1),
                            )
                        nc.scalar.copy(st["ke"][cht][:], ps[:])
                    return emit

                def vt_unit(op2):
                    def emit():
                        pp = p0 + op2 * 128
                        ps = pproj.tile([128, BLK], dt.float32, tag="proj", name=f"vtp{op2}_{blk}")
                        for cc in range(2):
                            nc.tensor.matmul(
                                ps[:, :256],
                                lhsT=xt_sb[:, cc, pp : pp + 128],
                                rhs=wv_sb[:, cc],
                                start=(cc == 0), stop=(cc == 1),
                            )
                        nc.scalar.copy(st["vt"][op2][:], ps[:, :256])
                    return emit

                def rope_unit(cht):
                    def emit():
                        nc.vector.tensor_mul(st["cq"][cht][:], st["q"][cht][:], ct_sb[:])
                        nc.vector.tensor_mul(st["sq"][cht][:], st["q"][cht][:], st_sb[:])
                    return emit

                def kprod_unit(dh):
                    def emit():
                        pk = kblk.tile([128, BLK], dt.bfloat16, tag="pk", name=f"pk{dh}_{blk}")
                        psw = kblk.tile([128, BLK], dt.bfloat16, tag="psw", name=f"psw{dh}_{blk}")
                        nc.vector.tensor_mul(pk[:], st["ke"][dh][:], ct_sb[:, :BLK])
                        nc.vector.tensor_mul(psw[:], st["ke"][2 + dh][:], st_sb[:, :BLK])
                        if dh == 0:
                            nc.vector.tensor_add(st["rk"][dh][:], pk[:], psw[:])
                        else:
                            nc.vector.tensor_sub(st["rk"][dh][:], pk[:], psw[:])
                        pk2 = kblk.tile([128, BLK], dt.bfloat16, tag="pk2", name=f"pk2{dh}_{blk}")
                        psw2 = kblk.tile([128, BLK], dt.bfloat16, tag="psw2", name=f"psw2{dh}_{blk}")
                        nc.vector.tensor_mul(pk2[:], st["ke"][dh][:], st_sb[:, :BLK])
                        nc.vector.tensor_mul(psw2[:], st["ke"][2 + dh][:], ct_sb[:, :BLK])
                        if dh == 0:
                            nc.vector.tensor_sub(st["rpk"][dh][:], pk2[:], psw2[:])
                        else:
                            nc.vector.tensor_add(st["rpk"][dh][:], pk2[:], psw2[:])
                    return emit

                units = [q_unit(m, cht) for m in range(M) for cht in range(2)]
                units.append(rope_unit(0))
                units.append(rope_unit(1))
                units += [k_unit(c) for c in range(4)]
                units.append(kprod_unit(0))
                units.append(kprod_unit(1))
                units += [vt_unit(i) for i in range(4)]
                return units

            def attn_units(blk):
                st = S[blk]
                units = []

                def qk_unit(op2p, quad):
                    def emit():
                        qkA = pqk.tile([128, 1024], dt.float32, tag="qk2", name=f"qkA{op2p}_{quad}_{blk}")
                        qkB = pqk.tile([128, 1024], dt.float32, tag="qk2", name=f"qkB{op2p}_{quad}_{blk}")
                        qkg = [qkA, qkA, qkB, qkB]
                        for g4 in range(4):
                            for op2l in range(2):
                                for oo in range(2):
                                    o = (op2p * 2 + op2l) * 2 + oo
                                    for ph, (kt, ut) in enumerate(((st["rk"], st["cq"]), (st["rpk"], st["sq"]))):
                                        foff = 512 * (g4 % 2) + 256 * op2l
                                        nc.tensor.matmul(
                                            qkg[g4][64 * oo : 64 * oo + 64, foff : foff + 256],
                                            lhsT=kt[quad][32 * g4 : 32 * g4 + 32, o * 64 : (o + 1) * 64],
                                            rhs=ut[quad][:]
                                            .rearrange("p (m t) -> p m t", m=M)[
                                                32 * g4 : 32 * g4 + 32, :, o * 64 : (o + 1) * 64
                                            ],
                                            start=(ph == 0), stop=(ph == 1),
                                            tile_position=(32 * g4, 64 * oo),
                                        )
                        w_sb = wpool.tile([128, 2048], dt.bfloat16, tag="w", name=f"w{op2p}_{quad}_{blk}")
                        nc.scalar.activation(w_sb[:, 0:1024], qkA[:], AF.Sigmoid, scale=SCALE)
                        nc.scalar.activation(w_sb[:, 1024:2048], qkB[:], AF.Sigmoid, scale=SCALE)
                        st[f"w{op2p}_{quad}"] = w_sb
                    return emit

                def av_unit(op2p, op2l):
                    def emit():
                        op2 = op2p * 2 + op2l
                        av = pqk.tile([128, 1024], dt.float32, tag="qk2", name=f"av{op2}_{blk}")
                        for quad in range(2):
                            for oo in range(2):
                                for g4 in range(4):
                                    woff = 1024 * (g4 // 2) + 512 * (g4 % 2) + 256 * op2l
                                    nc.tensor.matmul(
                                        av[32 * g4 : 32 * g4 + 32,
                                           512 * oo + 256 * quad : 512 * oo + 256 * quad + 256],
                                        lhsT=st["vt"][op2][
                                            64 * oo : 64 * oo + 64,
                                            32 * (quad * 4 + g4) : 32 * (quad * 4 + g4) + 32,
                                        ],
                                        rhs=st[f"w{op2p}_{quad}"][64 * oo : 64 * oo + 64, woff : woff + 256],
                                        start=True, stop=True,
                                        tile_position=(64 * oo, 32 * g4),
                                    )
                        av4 = av[:].rearrange("p (oo q mt) -> p oo q mt", oo=2, q=2)
                        for quad in range(2):
                            nc.vector.tensor_copy(
                                st["vc"][quad][:, op2 * 512 : (op2 + 1) * 512]
                                .rearrange("p (oo mt) -> p oo mt", oo=2),
                                av4[:, :, quad, :],
                            )
                    return emit

                for op2p in range(2):
                    units.append(qk_unit(op2p, 0))
                    units.append(qk_unit(op2p, 1))
                    units.append(av_unit(op2p, 0))
                    units.append(av_unit(op2p, 1))
                return units

            def outproj_units(blk):
                st = S[blk]

                def unit(och):
                    def emit():
                        ps = pproj.tile([128, BLK], dt.float32, tag="proj", name=f"ops{och}_{blk}")
                        for ch in range(8):
                            m, quad = ch // 2, ch % 2
                            nc.tensor.matmul(
                                ps[:],
                                lhsT=wo_sb[:, ch, och * 128 : (och + 1) * 128],
                                rhs=st["vc"][quad][:]
                                .rearrange("p (o m t) -> p o m t", o=8, m=M)[:, :, m, :],
                                start=(ch == 0), stop=(ch == 7),
                            )
                        o_sb = wpool.tile([128, BLK], dt.float32, tag="osb", name=f"osb{och}_{blk}")
                        nc.vector.tensor_copy(o_sb[:], ps[:])
                        nc.sync.dma_start(
                            out=outT[och * 128 : (och + 1) * 128, blk * BLK : (blk + 1) * BLK],
                            in_=o_sb[:],
                        )
                    return emit

                return [unit(0), unit(1)]

            def interleave(primary, fillers):
                """Emit primary units with filler units spread between them."""
                if not primary:
                    for f in fillers:
                        f()
                    return
                k = len(fillers)
                n = len(primary)
                fi = 0
                for i, p in enumerate(primary):
                    p()
                    take = (k * (i + 1)) // n - fi
                    for _ in range(take):
                        fillers[fi]()
                        fi += 1

            # prologue: block-0 projections, warming late tensors just in time
            p0units = proj_units(0)
            # unit order: 8 q-units, 2 rope, 4 k-units, 2 kprod, 4 vt-units
            for i, u in enumerate(p0units):
                if i == 4:
                    warm_touch(wq_sb[:, 0, 512:520]); warm_touch(wq_sb[:, 1, 512:520])
                if i == 10:
                    warm_touch(wk_sb[:, 0, 0:8]); warm_touch(wk_sb[:, 1, 0:8])
                if i == 16:
                    warm_touch(wv_sb[:, 0, 0:8]); warm_touch(wv_sb[:, 1, 0:8])
                u()
            warm_touch(wo_sb[:, 0, 0:8])
            warm_touch(ct_sb[:, 0:8]); warm_touch(st_sb[:, 0:8])
            for blk in range(NBLK):
                fillers = []
                if blk + 1 < NBLK:
                    for cc in range(2):
                        warm_touch(xt_sb[:, cc, (blk + 1) * BLK : (blk + 1) * BLK + 8])
                    fillers += proj_units(blk + 1)
                if blk - 1 >= 0:
                    fillers += outproj_units(blk - 1)
                interleave(attn_units(blk), fillers)
            for u in outproj_units(NBLK - 1):
                u()

    nc.compile()
    return nc


# ---------------------------------------------------------------- host side
def _rope_tables(pos_arr, rope_freq):
    scaling = np.pi / np.stack([np.linspace(1, 30, HH), np.linspace(0.1, 1, HH)], -1)
    freq = rope_freq * scaling.astype(np.float32)
    phi = (pos_arr[:, None, :] * freq[None, :, :]).sum(-1)   # [64, HH]
    cs, sn = np.cos(phi), np.sin(phi)
    Ct = np.repeat(cs.T, 2, axis=0).astype(np.float32)        # [32, 64] rows h
    St = np.repeat(sn.T, 2, axis=0).astype(np.float32)
    return np.tile(Ct, (4, 32)), np.tile(St, (4, 32))         # [128, 2048]


def _build_weights(Wq, Wk, Wv, Wo, axis):
    Wq_a = Wq[:, 2 * axis : 2 * axis + 2]                     # [C, d, v, M, KG, H]
    wq = np.transpose(Wq_a, (0, 3, 1, 2, 4, 5)).reshape(C, 1024)  # (m, d, v, kg, h)
    Wk_a = Wk[:, 2 * axis : 2 * axis + 2].reshape(C, 2, 2, KG, HH, 2)
    Wk_swap = np.stack([Wk_a[..., 1], -Wk_a[..., 0]], -1)
    wk = np.concatenate([Wk_a.reshape(C, 256), Wk_swap.reshape(C, 256)], 1)  # (e,d,v,kg,h)
    wv = (Wv[:, 2 * axis : 2 * axis + 2].reshape(C, 256) * DEN).astype(np.float32)
    Wo_a = Wo[2 * axis : 2 * axis + 2]                        # [d, v, M, KG, HV, C]
    Wo_perm = np.transpose(Wo_a, (2, 0, 1, 3, 4, 5)).reshape(M * 256, C)  # (m, c, f)
    wo = Wo_perm.reshape(8, 128, 256).transpose(1, 0, 2).reshape(128, 8 * 256)
    import ml_dtypes
    return (np.ascontiguousarray(wq).astype(ml_dtypes.bfloat16),
            np.ascontiguousarray(wk).astype(ml_dtypes.bfloat16),
            np.ascontiguousarray(wv).astype(ml_dtypes.bfloat16),
            np.ascontiguousarray(wo).astype(ml_dtypes.bfloat16))


def prepare_in_maps(x, Wq, Wk, Wv, bv, Wo, rope_freq, ypos, xpos, mask):
    import ml_dtypes

    assert np.abs(bv).max() == 0.0, "kernel assumes bv == 0 (spec fill=zeros)"
    Ct0, St0 = _rope_tables(ypos, rope_freq)
    Ct1, St1 = _rope_tables(xpos, rope_freq)
    waxis = [_build_weights(Wq, Wk, Wv, Wo, a) for a in range(2)]
    in_maps = []
    for core in range(8):
        b, axis, half = core // 4, (core // 2) % 2, core % 2
        wq, wk, wv, wo = waxis[axis]
        if axis == 0:
            blkx = x[b, :, 32 * half : 32 * half + 32, :]     # [Y, 32, C]
            xT = np.transpose(blkx, (2, 1, 0)).reshape(C, NPOS)  # (c, o=x, t=y)
            Ct, St = Ct0, St0
        else:
            blkx = x[b, 32 * half : 32 * half + 32, :, :]     # [32, X, C]
            xT = np.transpose(blkx, (2, 0, 1)).reshape(C, NPOS)  # (c, o=y, t=x)
            Ct, St = Ct1, St1
        in_maps.append(
            dict(
                xT=np.ascontiguousarray(xT).astype(ml_dtypes.bfloat16),
                wq=wq, wk=wk, wv=wv, wo=wo,
                ctab=Ct.astype(ml_dtypes.bfloat16),
                stab=St.astype(ml_dtypes.bfloat16),
            )
        )
    return in_maps


def gather_output(results):
    out = np.zeros((B, Y, X, C), np.float32)
    for core in range(8):
        b, axis, half = core // 4, (core // 2) % 2, core % 2
        outT = results[core]["outT"].reshape(C, 32, 64)
        if axis == 0:
            out[b, :, 32 * half : 32 * half + 32, :] += np.transpose(outT, (2, 1, 0))
        else:
            out[b, 32 * half : 32 * half + 32, :, :] += np.transpose(outT, (1, 2, 0))
    return out


_CACHED = {}


def kernel(x, Wq, Wk, Wv, bv, Wo, rope_freq, ypos, xpos, mask):
    from concourse.bass_utils import run_bass_kernel_spmd

    x, Wq, Wk, Wv, bv, Wo, rope_freq, ypos, xpos = (
        np.asarray(a, np.float32) for a in (x, Wq, Wk, Wv, bv, Wo, rope_freq, ypos, xpos)
    )
    in_maps = prepare_in_maps(x, Wq, Wk, Wv, bv, Wo, rope_freq, ypos, xpos, mask)
    if "nc" not in _CACHED:
        _CACHED["nc"] = build_program()
    res = run_bass_kernel_spmd(_CACHED["nc"], in_maps, core_ids=list(range(8)))
    return gather_output(res.results)
